# revision 35
# baseline (speedup 1.0000x reference)
"""Trainium2 Bass kernel for nn_ActorTransformer (B=16, T=1024, D=96, L=6, NH=6).

Strategy: pure data parallelism, 2 batch elements per NeuronCore on 8 cores.
Inside a core, activations live transposed as [D=96 partitions, T=1024 free].
Attention is computed as attT [Tk, Tq] blocks so the AV matmul needs no
transposes; per-head matmuls are packed into 32-row/col PE tile groups and
run in bf16.

v2 changes vs baseline:
- softmax denominators are folded into the AV matmul as an extra all-ones
  column per head (V bands are 17 wide), killing the dedicated ones-lhsT
  denominator matmuls (~190us of PE time).
- causal trimming: for diagonal-crossing key tiles, QK/exp/AV only cover the
  q-column range that has any unmasked key (saves ~25% of attention work).
- the causal mask is applied only to the 128-col crossing band (in-place
  affine_select), or optionally as a triangle-constant accumulating matmul
  (MASK_TRI=1).
- scale 1/sqrt(hs) folded into the host-side Wq layout.
- per-iteration memsets (ocat, dP) removed; constant regions are written once.
"""

import os
import sys

import ml_dtypes
import numpy as np

sys.path.insert(0, "/opt/trn_rl_repo")

import concourse.bass as bass
import concourse.mybir as mybir
import concourse.tile as tile
from concourse import bacc
from concourse.bass_utils import run_bass_kernel_spmd
from concourse.masks import make_identity

# Model constants
L, NH, D, HS, FF = 6, 6, 96, 16, 384
DG, DO, DA = 32, 64, 16
T, B = 1024, 16
NCORES = 8
BPC = B // NCORES  # batch per core
SCALE = HS ** -0.5  # 0.25
EPS = 1e-5
CW = 512  # chunk width (matmul moving free dim)
NCH = T // CW  # 2
NT = T // 128  # 8 token tiles
F32 = mybir.dt.float32
BF16 = mybir.dt.bfloat16
NPBF = ml_dtypes.bfloat16
ATT_F32 = os.environ.get("ATT_F32", "0") == "1"
EDT = F32 if ATT_F32 else BF16
MASK_TRI = os.environ.get("MASK_TRI", "0") == "1"
NEG = -60000.0

LAST_RESULT = None


def _dt(x):
    return np.ascontiguousarray(np.asarray(x, dtype=np.float32))


def build(nc):
    """Builds the full forward pass for BPC batch elements on one core."""
    def din(name, shape, dt=F32):
        return nc.dram_tensor(name, list(shape), dt, kind="ExternalInput").ap()

    goals = din("goals", [BPC, 1, DG])
    obss = din("obss", [BPC, T, DO])
    w_go = din("w_go", [DG + DO, D])
    b_go = din("b_go", [D])
    posT_d = din("posT", [D, T])
    wqa = din("wqa", [L, D, 112], BF16)
    wqb = din("wqb", [L, D, 48], BF16)
    wka = din("wka", [L, D, 112], BF16)
    wkb = din("wkb", [L, D, 48], BF16)
    wv_hd = din("wv_hd", [L, D + 1, NH * 32], BF16)
    wpa = din("wpa", [L, 128, D], BF16)
    wpb = din("wpb", [L, 64, D], BF16)
    wf1 = din("wf1", [L, D, FF], BF16)
    wf2 = din("wf2", [L, FF, D], BF16)
    wact_d = din("wact", [D, DA], BF16)
    ln1_g = din("ln1_g", [L, D])
    ln1_b = din("ln1_b", [L, D])
    ln2_g = din("ln2_g", [L, D])
    ln2_b = din("ln2_b", [L, D])
    b_proj = din("b_proj", [L, D])
    b_ff1 = din("b_ff1", [L, FF])
    b_ff2 = din("b_ff2", [L, D])
    lnf_g = din("lnf_g", [D])
    lnf_b = din("lnf_b", [D])
    b_act = din("b_act", [DA])
    selA_d = din("selA", [128, 128], BF16)
    selB_d = din("selB", [64, 64], BF16)
    out = nc.dram_tensor("out", [BPC, T, DA], F32, kind="ExternalOutput").ap()

    with tile.TileContext(nc) as tc:
        _body(tc, goals, obss, w_go, b_go, posT_d, wqa, wqb, wka, wkb, wv_hd,
              wpa, wpb, wf1, wf2, wact_d, ln1_g, ln1_b, ln2_g, ln2_b,
              b_proj, b_ff1, b_ff2, lnf_g, lnf_b, b_act, selA_d, selB_d, out)
    return nc


def _body(tc, goals, obss, w_go, b_go, posT_d, wqa, wqb, wka, wkb, wv_hd,
          wpa, wpb, wf1, wf2, wact_d, ln1_g, ln1_b, ln2_g, ln2_b,
          b_proj, b_ff1, b_ff2, lnf_g, lnf_b, b_act, selA_d, selB_d, out):
    from contextlib import ExitStack
    nc = tc.nc
    ctx = ExitStack()

    wp = ctx.enter_context(tc.tile_pool(name="wp", bufs=1))     # weights/consts
    xp = ctx.enter_context(tc.tile_pool(name="xp", bufs=1))     # residual streams
    tp = ctx.enter_context(tc.tile_pool(name="tp", bufs=2))     # transient small
    xnp = ctx.enter_context(tc.tile_pool(name="xnp", bufs=2))   # LN outputs
    qkp = ctx.enter_context(tc.tile_pool(name="qkp", bufs=2))   # q/k spreads
    vp = ctx.enter_context(tc.tile_pool(name="vp", bufs=2))     # v tiles
    ep = ctx.enter_context(tc.tile_pool(name="ep", bufs=26))    # exp(att) blocks
    op_ = ctx.enter_context(tc.tile_pool(name="op", bufs=2))    # attn out concat
    hp = ctx.enter_context(tc.tile_pool(name="hp", bufs=2))     # ff hidden
    pp = ctx.enter_context(tc.tile_pool(name="pp", bufs=8, space="PSUM"))

    def psum(p, f, tag="ps"):
        return pp.tile([p, f], F32, tag=tag, name=tag)

    # ---------------- constants ----------------
    ident = wp.tile([128, 128], F32, tag="ident")
    make_identity(nc, ident[:])
    ones96 = wp.tile([96, 1], F32, tag="ones96")
    nc.vector.memset(ones96[:], 1.0)
    ones96b = wp.tile([96, 1], BF16, tag="ones96b")
    nc.vector.memset(ones96b[:], 1.0)
    eps_c = wp.tile([1, 1], F32, tag="eps_c")
    nc.vector.memset(eps_c[:], EPS)

    if MASK_TRI:
        # tri[x, y] = NEG if x < y else 0 ; idm = [I128 | 0] (bf16)
        tri = wp.tile([128, 128], BF16, tag="tri")
        nc.gpsimd.memset(tri[:], NEG)
        nc.gpsimd.affine_select(
            out=tri[:], in_=tri[:], pattern=[[-1, 128]],
            compare_op=mybir.AluOpType.is_lt, fill=0.0, base=0,
            channel_multiplier=1)
        idm = wp.tile([128, 128], BF16, tag="idm")
        make_identity(nc, idm[:])

    # b_act broadcast to [128, DA] (free-dim bias for the final head)
    bact_row = wp.tile([1, DA], F32, tag="bact_row")
    nc.sync.dma_start(out=bact_row[:], in_=b_act[None, :])
    bact_b = wp.tile([128, DA], F32, tag="bact_b")
    nc.gpsimd.partition_broadcast(bact_b[:], bact_row[:])

    posT = wp.tile([D, T], F32, tag="posT")
    nc.sync.dma_start(out=posT[:], in_=posT_d[:, :])

    wgo_g = wp.tile([DG, D], F32, tag="wgo_g")
    nc.sync.dma_start(out=wgo_g[:], in_=w_go[0:DG, :])
    wgo_o = wp.tile([DO, D], F32, tag="wgo_o")
    nc.sync.dma_start(out=wgo_o[:], in_=w_go[DG:DG + DO, :])
    bgo = wp.tile([D, 1], F32, tag="bgo")
    nc.sync.dma_start(out=bgo[:], in_=b_go[:, None])

    # per-layer weights: direct bf16 DMAs (host pre-laid-out)
    WQA, WQB, WKA, WKB, WV, WP, WF1, WF2 = [], [], [], [], [], [], [], []
    L1G, L1B, L2G, L2B, BP, BF1, BF2 = [], [], [], [], [], [], []

    def wtile(src, shape, tag, dt=BF16):
        t_ = wp.tile(list(shape), dt, tag=tag, name=tag)
        nc.sync.dma_start(out=t_[:], in_=src)
        return t_

    for l in range(L):
        WQA.append(wtile(wqa[l], [D, 112], f"wqA{l}"))
        WQB.append(wtile(wqb[l], [D, 48], f"wqB{l}"))
        WKA.append(wtile(wka[l], [D, 112], f"wkA{l}"))
        WKB.append(wtile(wkb[l], [D, 48], f"wkB{l}"))
        WV.append(wtile(wv_hd[l], [D + 1, NH * 32], f"wv{l}"))
        WP.append((wtile(wpa[l], [128, D], f"wpA{l}"),
                   wtile(wpb[l], [64, D], f"wpB{l}")))
        WF1.append(wtile(wf1[l], [D, FF], f"wf1{l}"))
        WF2.append([wtile(wf2[l, 128 * f:128 * (f + 1), :], [128, D], f"wf2{l}_{f}")
                    for f in range(3)])

        def colv(src, tag, p=D):
            t_ = wp.tile([p, 1], F32, tag=tag, name=tag)
            nc.sync.dma_start(out=t_[:], in_=src[:, None])
            return t_
        L1G.append(colv(ln1_g[l], f"l1g{l}"))
        L1B.append(colv(ln1_b[l], f"l1b{l}"))
        L2G.append(colv(ln2_g[l], f"l2g{l}"))
        L2B.append(colv(ln2_b[l], f"l2b{l}"))
        BP.append(colv(b_proj[l], f"bp{l}"))
        BF2.append(colv(b_ff2[l], f"bf2{l}"))
        BF1.append([colv(b_ff1[l, 128 * f:128 * (f + 1)], f"bf1{l}_{f}", p=128)
                    for f in range(3)])

    lnfg = wp.tile([D, 1], F32, tag="lnfg")
    nc.sync.dma_start(out=lnfg[:], in_=lnf_g[:, None])
    lnfb = wp.tile([D, 1], F32, tag="lnfb")
    nc.sync.dma_start(out=lnfb[:], in_=lnf_b[:, None])
    wact = wtile(wact_d[:, :], [D, DA], "wact")
    selA = wtile(selA_d[:, :], [128, 128], "selA")
    selB = wtile(selB_d[:, :], [64, 64], "selB")

    # ---------------- embedding ----------------
    XT = []
    obsp_cm = tc.tile_pool(name="obsp", bufs=1)
    obsp = obsp_cm.__enter__()
    for b in range(BPC):
        with nc.named_scope(f"embed{b}"):
            obsT = obsp.tile([DO, T], F32, tag=f"obsT{b}", name=f"obsT{b}")
            for t in range(NT):
                otile = tp.tile([128, DO], F32, tag="otile")
                nc.sync.dma_start(out=otile[:], in_=obss[b, 128 * t:128 * (t + 1), :])
                ps = psum(DO, 128)
                nc.tensor.transpose(ps[:], otile[:], ident[:])
                nc.vector.tensor_copy(out=obsT[:, 128 * t:128 * (t + 1)], in_=ps[:])
            gT = tp.tile([DG, 1], F32, tag="gT")
            nc.sync.dma_start(out=gT[:], in_=goals[b, 0, :, None])
            cps = psum(D, 1)
            nc.tensor.matmul(cps[:], wgo_g[:], gT[:], start=True, stop=True)
            cgo = tp.tile([D, 1], F32, tag="cgo")
            nc.vector.tensor_add(out=cgo[:], in0=cps[:], in1=bgo[:])
            xT = xp.tile([D, T], F32, tag=f"xT{b}")
            for c in range(NCH):
                sl = slice(CW * c, CW * (c + 1))
                ps = psum(D, CW)
                nc.tensor.matmul(ps[:], wgo_o[:], obsT[:, sl],
                                 start=True, stop=True)
                nc.vector.tensor_scalar(out=xT[:, sl], in0=ps[:], scalar1=cgo[:],
                                        scalar2=None, op0=mybir.AluOpType.add)
                nc.vector.tensor_add(out=xT[:, sl], in0=xT[:, sl], in1=posT[:, sl])
            XT.append(xT)
    obsp_cm.__exit__(None, None, None)

    # ---------------- helpers ----------------
    def layernorm(xT, g, b, tag):
        """x [D, T] -> (x - mean)/sqrt(var+eps) * g + b, stats over D (partitions).

        Works on a bf16 copy of x: the stats matmuls stream bf16 (4x faster
        than fp32 on the PE) and the elementwise passes run at 2x DVE rate.
        The xn tile has one spare row (D) used by the caller as a constant-ones
        contraction row for the V matmul."""
        xn = xnp.tile([D + 1, T], BF16, tag=tag, name=tag)
        for c in range(NCH):
            sl = slice(CW * c, CW * (c + 1))
            xb = tp.tile([D, CW], BF16, tag="xb")
            nc.vector.tensor_copy(out=xb[:], in_=xT[:, sl])
            x2c = tp.tile([D, CW], BF16, tag="x2c")
            nc.vector.tensor_mul(out=x2c[:], in0=xb[:], in1=xb[:])
            ps1 = psum(1, CW)
            nc.tensor.matmul(ps1[:], ones96b[:], xb[:], start=True, stop=True)
            ps2 = psum(1, CW)
            nc.tensor.matmul(ps2[:], ones96b[:], x2c[:], start=True, stop=True)
            # m = s1/96 ; var = s2/96 - m^2 ; rstd = exp(-0.5*ln(var+eps))
            m_sb = tp.tile([1, CW], F32, tag="m_sb")
            nc.vector.tensor_scalar(out=m_sb[:], in0=ps1[:], scalar1=1.0 / D,
                                    scalar2=None, op0=mybir.AluOpType.mult)
            msq = tp.tile([1, CW], F32, tag="msq")
            nc.vector.tensor_mul(out=msq[:], in0=m_sb[:], in1=m_sb[:])
            var = tp.tile([1, CW], F32, tag="var")
            nc.vector.tensor_scalar(out=var[:], in0=ps2[:], scalar1=1.0 / D,
                                    scalar2=None, op0=mybir.AluOpType.mult)
            nc.vector.tensor_sub(out=var[:], in0=var[:], in1=msq[:])
            nc.scalar.activation(out=var[:], in_=var[:],
                                 func=mybir.ActivationFunctionType.Ln,
                                 bias=eps_c[:])
            rs_sb = tp.tile([1, CW], F32, tag="rs_sb")
            nc.scalar.activation(out=rs_sb[:], in_=var[:],
                                 func=mybir.ActivationFunctionType.Exp, scale=-0.5)
            m_sbh = tp.tile([1, CW], BF16, tag="m_sbh")
            nc.vector.tensor_copy(out=m_sbh[:], in_=m_sb[:])
            rs_sbh = tp.tile([1, CW], BF16, tag="rs_sbh")
            nc.vector.tensor_copy(out=rs_sbh[:], in_=rs_sb[:])
            m_b = tp.tile([D, CW], BF16, tag="m_b")
            rs_b = tp.tile([D, CW], BF16, tag="rs_b")
            nc.gpsimd.partition_broadcast(m_b[:], m_sbh[:])
            nc.gpsimd.partition_broadcast(rs_b[:], rs_sbh[:])
            xnf = tp.tile([D, CW], BF16, tag="xnf")
            nc.vector.tensor_sub(out=xnf[:], in0=xb[:], in1=m_b[:])
            nc.vector.tensor_mul(out=xnf[:], in0=xnf[:], in1=rs_b[:])
            nc.vector.tensor_scalar(out=xn[0:D, sl], in0=xnf[:], scalar1=g[:],
                                    scalar2=b[:], op0=mybir.AluOpType.mult,
                                    op1=mybir.AluOpType.add)
        return xn

    # ---------------- transformer layers ----------------
    # The two batch elements are interleaved phase-by-phase so that while the
    # Scalar engine grinds b0's exps the PE streams b1's matmuls (and vice
    # versa): no engine waits on a single serial chain, and the PE stays busy
    # enough to hold its warm (2.4 GHz) clock.
    for l in range(L):
        XN1, SPR, VG = [], [], []
        for b in range(BPC):
            with nc.named_scope(f"L{l}b{b}.ln1"):
                XN1.append(layernorm(XT[b], L1G[l], L1B[l], "xn1"))
        for b in range(BPC):
            xn = XN1[b]
            with nc.named_scope(f"L{l}b{b}.qkv"):
                # q/k spread tiles: head h<4 at rows 32h of A, h>=4 at 32(h-4) of B
                qA = qkp.tile([112, T], BF16, tag="qA")
                qB = qkp.tile([48, T], BF16, tag="qB")
                kA = qkp.tile([112, T], BF16, tag="kA")
                kB = qkp.tile([48, T], BF16, tag="kB")
                # constant-ones contraction row for the V matmul: makes WV's
                # ones-block produce 1.0 columns so each 32-wide vaug band is
                # [16 ones cols | 16 V cols] with no strided copies.
                nc.vector.memset(xn[D:D + 1, :], 1.0)
                for c in range(NCH):
                    sl = slice(CW * c, CW * (c + 1))
                    for dst, w in ((qA, WQA[l]), (qB, WQB[l]),
                                   (kA, WKA[l]), (kB, WKB[l])):
                        p = dst.shape[0]
                        ps = psum(p, CW)
                        nc.tensor.matmul(ps[:], w[:], xn[0:D, sl],
                                         start=True, stop=True)
                        nc.vector.tensor_copy(out=dst[:, sl], in_=ps[:])
                # vaug: per key-tile t, head h at cols 192t+32h: 16 all-ones
                # cols then 16 V cols, so the 32-wide AV matmul band also
                # produces the softmax denominator replicated on psum rows
                # col..col+16 (row col is 32-aligned)
                vaug = vp.tile([128, 192 * NT], EDT, tag="vaug")
                for t in range(NT):
                    ps = psum(128, NH * 32)
                    nc.tensor.matmul(ps[:], xn[0:D + 1, 128 * t:128 * (t + 1)],
                                     WV[l][:], start=True, stop=True)
                    nc.vector.tensor_copy(out=vaug[:, 192 * t:192 * (t + 1)],
                                          in_=ps[:])
                SPR.append((qA, qB, kA, kB))
                VG.append(vaug)
        OC = [(op_.tile([128, T], BF16, tag="ocatA", name="ocatA"),
               op_.tile([64, T], BF16, tag="ocatB", name="ocatB"))
              for b in range(BPC)]
        for c in range(NCH):
            smax = 4 * c + 3
            oX = [(psum(128, CW), psum(64, CW)) for b in range(BPC)]
            pend = [[] for b in range(BPC)]

            def flush_av(b, lim):
                # AV matmuls trail the QK/exp stream by a whole key-step: the
                # PE never waits on exp, and QK (row-tiled) / AV (col-tiled)
                # run as same-mode bursts.
                oA, oB = oX[b]
                vaug = VG[b]
                while len(pend[b]) > lim:
                    fs, h, et, n0 = pend[b].pop(0)
                    o, col = (oA, 32 * h) if h < 4 else (oB, 32 * (h - 4))
                    nc.tensor.matmul(
                        o[col:col + 32, n0:CW],
                        vaug[:, 192 * fs + 32 * h:192 * fs + 32 * h + 32],
                        et[:, n0:CW],
                        start=(fs == 0), stop=(fs == smax),
                        tile_position=(0, col), skip_group_check=True)

            for s in range(smax + 1):
                crossed = s >= 4 * c
                n0 = 128 * s - CW * c if crossed else 0
                for b in range(BPC):
                    qA, qB, kA, kB = SPR[b]
                    with nc.named_scope(f"L{l}b{b}.att{c}"):
                        for h in range(NH):
                            spr_k, spr_q, row = ((kA, qA, 32 * h) if h < 4
                                                 else (kB, qB, 32 * (h - 4)))
                            pqk = psum(128, CW)
                            nc.tensor.matmul(
                                pqk[:, 0:CW - n0],
                                spr_k[row:row + HS, 128 * s:128 * (s + 1)],
                                spr_q[row:row + HS, CW * c + n0:CW * (c + 1)],
                                start=True, stop=not (crossed and MASK_TRI),
                                tile_position=(row, 0),
                                skip_group_check=True)
                            if crossed and MASK_TRI:
                                nc.tensor.matmul(
                                    pqk[:, 0:128], tri[:], idm[:],
                                    start=False, stop=True,
                                    tile_position=(0, 0), skip_group_check=True)
                            et = ep.tile([128, CW], EDT, tag="et")
                            if crossed and not MASK_TRI:
                                # diagonal band: exp into a scratch tile, then
                                # causal-select into et; rest exps directly
                                if CW - n0 > 128:
                                    nc.scalar.activation(
                                        out=et[:, n0 + 128:CW],
                                        in_=pqk[:, 128:CW - n0],
                                        func=mybir.ActivationFunctionType.Exp)
                                etb = ep.tile([128, 128], EDT, tag="etb",
                                              bufs=12)
                                nc.scalar.activation(
                                    out=etb[:], in_=pqk[:, 0:128],
                                    func=mybir.ActivationFunctionType.Exp)
                                nc.gpsimd.affine_select(
                                    out=et[:, n0:n0 + 128], in_=etb[:],
                                    pattern=[[1, 128]],
                                    compare_op=mybir.AluOpType.is_ge,
                                    fill=0.0, base=0, channel_multiplier=-1)
                            else:
                                nc.scalar.activation(
                                    out=et[:, n0:CW], in_=pqk[:, 0:CW - n0],
                                    func=mybir.ActivationFunctionType.Exp)
                            pend[b].append((s, h, et, n0))
                for b in range(BPC):
                    with nc.named_scope(f"L{l}b{b}.att{c}"):
                        flush_av(b, NH if s < smax else 0)
            for b in range(BPC):
                oA, oB = oX[b]
                ocatA, ocatB = OC[b]
                with nc.named_scope(f"L{l}b{b}.attep{c}"):
                    # softmax denominators sit (replicated) on psum rows
                    # col..col+16 of each band. Broadcast across the full
                    # 32-row band with a constant selection matmul (sel[k,p]=1
                    # iff k == 32*(p//32)) via an SBUF bounce copy, reciprocal,
                    # then scale. Rows col..col+16 of each ocat band become
                    # sigma/sigma = 1.0 and hit zero rows of w_proj downstream.
                    sden = tp.tile([128, CW], BF16, tag="sden")
                    sdenB = tp.tile([64, CW], BF16, tag="sdenB")
                    nc.vector.tensor_copy(out=sden[:], in_=oA[:])
                    nc.vector.tensor_copy(out=sdenB[:], in_=oB[:])
                    sbA = psum(128, CW)
                    nc.tensor.matmul(sbA[:], selA[:], sden[:],
                                     start=True, stop=True)
                    sbB = psum(64, CW)
                    nc.tensor.matmul(sbB[:], selB[:], sdenB[:],
                                     start=True, stop=True)
                    rbr = tp.tile([128, CW], F32, tag="rbr")
                    rbrB = tp.tile([64, CW], F32, tag="rbrB")
                    nc.vector.reciprocal_approx_fast(out=rbr[:], in_=sbA[:])
                    nc.vector.reciprocal_approx_fast(out=rbrB[:], in_=sbB[:])
                    for h in range(NH):
                        o, col = (oA, 32 * h) if h < 4 else (oB, 32 * (h - 4))
                        oc = ocatA if h < 4 else ocatB
                        rt = rbr if h < 4 else rbrB
                        nc.vector.tensor_mul(
                            out=oc[col:col + 32, CW * c:CW * (c + 1)],
                            in0=o[col:col + 32, :], in1=rt[col:col + 32, :])
        for b in range(BPC):
            xT = XT[b]
            ocatA, ocatB = OC[b]
            with nc.named_scope(f"L{l}b{b}.proj"):
                for c in range(NCH):
                    sl = slice(CW * c, CW * (c + 1))
                    ps = psum(D, CW)
                    nc.tensor.matmul(ps[:], WP[l][0][:], ocatA[:, sl],
                                     start=True, stop=False)
                    nc.tensor.matmul(ps[:], WP[l][1][:], ocatB[:, sl],
                                     start=False, stop=True)
                    nc.vector.tensor_scalar(out=ps[:], in0=ps[:], scalar1=BP[l][:],
                                            scalar2=None, op0=mybir.AluOpType.add)
                    nc.vector.tensor_add(out=xT[:, sl], in0=xT[:, sl], in1=ps[:])
        XN2 = []
        for b in range(BPC):
            with nc.named_scope(f"L{l}b{b}.ln2"):
                XN2.append(layernorm(XT[b], L2G[l], L2B[l], "xn2"))
        for b in range(BPC):
            xT = XT[b]
            xn2 = XN2[b]
            with nc.named_scope(f"L{l}b{b}.ff"):
                h1 = [hp.tile([128, T], BF16, tag=f"h1_{f}", name=f"h1_{f}")
                      for f in range(3)]
                for c in range(NCH):
                    sl = slice(CW * c, CW * (c + 1))
                    for f in range(3):
                        ps = psum(128, CW)
                        nc.tensor.matmul(ps[:], WF1[l][:, 128 * f:128 * (f + 1)],
                                         xn2[0:D, sl], start=True, stop=True)
                        nc.vector.tensor_scalar(
                            out=h1[f][:, sl], in0=ps[:], scalar1=BF1[l][f][:],
                            scalar2=0.0, op0=mybir.AluOpType.add,
                            op1=mybir.AluOpType.max)
                for c in range(NCH):
                    sl = slice(CW * c, CW * (c + 1))
                    ps = psum(D, CW)
                    for f in range(3):
                        nc.tensor.matmul(ps[:], WF2[l][f][:], h1[f][:, sl],
                                         start=(f == 0), stop=(f == 2))
                    nc.vector.tensor_scalar(out=ps[:], in0=ps[:], scalar1=BF2[l][:],
                                            scalar2=None, op0=mybir.AluOpType.add)
                    nc.vector.tensor_add(out=xT[:, sl], in0=xT[:, sl], in1=ps[:])

    # ---------------- final LN + head ----------------
    for b in range(BPC):
        with nc.named_scope(f"head{b}"):
            xnf = layernorm(XT[b], lnfg, lnfb, "xn1")
            for t in range(NT):
                ps = psum(128, DA)
                nc.tensor.matmul(ps[:], xnf[0:D, 128 * t:128 * (t + 1)], wact[:],
                                 start=True, stop=True)
                osb = tp.tile([128, DA], F32, tag="osb")
                nc.vector.tensor_add(out=osb[:], in0=ps[:], in1=bact_b[:])
                nc.sync.dma_start(out=out[b, 128 * t:128 * (t + 1), :], in_=osb[:])

    ctx.close()


_CACHED = None


def _pin_act_tables():
    # Pin every ACT function to the one set containing both Ln and Exp so the
    # table is loaded once instead of thrashing between sets per activation.
    from concourse import hw_specs
    import concourse.bacc as bacc_mod
    if getattr(hw_specs, "_act_tables_pinned", False):
        return
    orig = hw_specs.get_activation_tables

    def pinned(arch):
        t = orig(arch)
        keep = "natural_log_exp_and_others"
        return {n: (f if n == keep else set()) for n, f in t.items()}

    hw_specs.get_activation_tables = pinned
    if hasattr(bacc_mod, "get_activation_tables"):
        bacc_mod.get_activation_tables = pinned
    hw_specs._act_tables_pinned = True


def _get_nc():
    global _CACHED
    if _CACHED is None:
        _pin_act_tables()
        nc = bacc.Bacc("TRN2", target_bir_lowering=False, debug=False,
                       enable_asserts=False)
        build(nc)
        nc.compile()
        _CACHED = nc
    return _CACHED


def prep_weights(inputs):
    """Host-side layout/casting of the (replicated) weights."""
    f = {k: _dt(v) for k, v in inputs.items()}
    o = {}
    o["w_go"] = f["w_go"]
    o["b_go"] = f["b_go"]
    o["posT"] = np.ascontiguousarray(f["pos_emb"].T)
    wq, wk, wv, wpj = f["wq"], f["wk"], f["wv"], f["w_proj"]
    qa = np.zeros((L, D, 112), np.float32)
    qb = np.zeros((L, D, 48), np.float32)
    ka = np.zeros((L, D, 112), np.float32)
    kb = np.zeros((L, D, 48), np.float32)
    pa = np.zeros((L, 128, D), np.float32)
    pb = np.zeros((L, 64, D), np.float32)
    for h in range(NH):
        if h < 4:
            qa[:, :, 32 * h:32 * h + HS] = wq[:, h] * SCALE
            ka[:, :, 32 * h:32 * h + HS] = wk[:, h]
            # ocat rows col..col+16 are sigma/sigma = 1.0; o sits at +16
            pa[:, 32 * h + HS:32 * h + 32, :] = wpj[:, HS * h:HS * (h + 1), :]
        else:
            r = 32 * (h - 4)
            qb[:, :, r:r + HS] = wq[:, h] * SCALE
            kb[:, :, r:r + HS] = wk[:, h]
            pb[:, r + HS:r + 32, :] = wpj[:, HS * h:HS * (h + 1), :]
    o["wqa"] = qa.astype(NPBF)
    o["wqb"] = qb.astype(NPBF)
    o["wka"] = ka.astype(NPBF)
    o["wkb"] = kb.astype(NPBF)
    # wv_hd [L, D+1, NH*32]: per head, cols [32h:32h+16) come from the
    # constant-ones xn row (row D) -> all-ones output; cols [32h+16:32h+32)
    # are the V projection.
    wvp = np.zeros((L, D + 1, NH * 32), np.float32)
    for h in range(NH):
        wvp[:, 0:D, 32 * h + HS:32 * h + 32] = wv[:, h]
        wvp[:, D, 32 * h:32 * h + HS] = 1.0
    o["wv_hd"] = wvp.astype(NPBF)
    o["wpa"] = pa.astype(NPBF)
    o["wpb"] = pb.astype(NPBF)
    o["wf1"] = f["w_ff1"].astype(NPBF)
    o["wf2"] = f["w_ff2"].astype(NPBF)
    o["wact"] = f["w_act"].astype(NPBF)
    sa = np.zeros((128, 128), np.float32)
    sa[32 * (np.arange(128) // 32), np.arange(128)] = 1.0
    o["selA"] = sa.astype(NPBF)
    o["selB"] = np.ascontiguousarray(sa[:64, :64]).astype(NPBF)
    for k in ("ln1_g", "ln1_b", "ln2_g", "ln2_b", "b_proj", "b_ff1", "b_ff2",
              "lnf_g", "lnf_b", "b_act"):
        o[k] = f[k]
    return o, f


def kernel(**inputs):
    global LAST_RESULT
    nc = _get_nc()
    w, f = prep_weights(inputs)
    in_maps = []
    for i in range(NCORES):
        m = dict(w)
        m["goals"] = f["goals"][BPC * i:BPC * (i + 1)]
        m["obss"] = f["obss"][BPC * i:BPC * (i + 1)]
        in_maps.append(m)
    res = run_bass_kernel_spmd(nc, in_maps, core_ids=list(range(NCORES)))
    LAST_RESULT = res
    return np.concatenate([r["out"] for r in res.results], axis=0)


# revision 45
# speedup vs baseline: 1.3538x; 1.3538x over previous
"""Trainium2 Bass kernel for nn_ActorTransformer (B=16, T=1024, D=96, L=6, NH=6).

Strategy: pure data parallelism, 2 batch elements per NeuronCore on 8 cores.
Inside a core, activations live transposed as [D=96 partitions, T=1024 free].
Attention is computed as attT [Tk, Tq] blocks so the AV matmul needs no
transposes; per-head matmuls are packed into 32-row/col PE tile groups and
run in bf16.

v2 changes vs baseline:
- softmax denominators are folded into the AV matmul as an extra all-ones
  column per head (V bands are 17 wide), killing the dedicated ones-lhsT
  denominator matmuls (~190us of PE time).
- causal trimming: for diagonal-crossing key tiles, QK/exp/AV only cover the
  q-column range that has any unmasked key (saves ~25% of attention work).
- the causal mask is applied only to the 128-col crossing band (in-place
  affine_select), or optionally as a triangle-constant accumulating matmul
  (MASK_TRI=1).
- scale 1/sqrt(hs) folded into the host-side Wq layout.
- per-iteration memsets (ocat, dP) removed; constant regions are written once.
"""

import os
import sys

import ml_dtypes
import numpy as np

sys.path.insert(0, "/opt/trn_rl_repo")

import concourse.bass as bass
import concourse.mybir as mybir
import concourse.tile as tile
from concourse import bacc
from concourse.bass_utils import run_bass_kernel_spmd
from concourse.masks import make_identity

# Model constants
L, NH, D, HS, FF = 6, 6, 96, 16, 384
DG, DO, DA = 32, 64, 16
T, B = 1024, 16
NCORES = 8
BPC = B // NCORES  # batch per core
SCALE = HS ** -0.5  # 0.25
EPS = 1e-5
CW = 512  # chunk width (matmul moving free dim)
NCH = T // CW  # 2
NT = T // 128  # 8 token tiles
F32 = mybir.dt.float32
BF16 = mybir.dt.bfloat16
NPBF = ml_dtypes.bfloat16
ATT_F32 = os.environ.get("ATT_F32", "0") == "1"
EDT = F32 if ATT_F32 else BF16
MASK_TRI = os.environ.get("MASK_TRI", "0") == "1"
NEG = -60000.0

LAST_RESULT = None


def _dt(x):
    return np.ascontiguousarray(np.asarray(x, dtype=np.float32))


def build(nc):
    """Builds the full forward pass for BPC batch elements on one core."""
    def din(name, shape, dt=F32):
        return nc.dram_tensor(name, list(shape), dt, kind="ExternalInput").ap()

    goals = din("goals", [BPC, 1, DG])
    obss = din("obss", [BPC, T, DO])
    w_go = din("w_go", [DG + DO, D])
    b_go = din("b_go", [D])
    posT_d = din("posT", [D, T])
    wqa = din("wqa", [L, D, 112], BF16)
    wqb = din("wqb", [L, D, 48], BF16)
    wka = din("wka", [L, D, 112], BF16)
    wkb = din("wkb", [L, D, 48], BF16)
    wv_hd = din("wv_hd", [L, D + 1, NH * 32], BF16)
    wpa = din("wpa", [L, 128, D], BF16)
    wpb = din("wpb", [L, 64, D], BF16)
    wf1 = din("wf1", [L, D, FF], BF16)
    wf2 = din("wf2", [L, FF, D], BF16)
    wact_d = din("wact", [D, DA], BF16)
    ln1_g = din("ln1_g", [L, D])
    ln1_b = din("ln1_b", [L, D])
    ln2_g = din("ln2_g", [L, D])
    ln2_b = din("ln2_b", [L, D])
    b_proj = din("b_proj", [L, D])
    b_ff1 = din("b_ff1", [L, FF])
    b_ff2 = din("b_ff2", [L, D])
    lnf_g = din("lnf_g", [D])
    lnf_b = din("lnf_b", [D])
    b_act = din("b_act", [DA])
    selA_d = din("selA", [128, 128], BF16)
    selB_d = din("selB", [64, 64], BF16)
    out = nc.dram_tensor("out", [BPC, T, DA], F32, kind="ExternalOutput").ap()

    with tile.TileContext(nc) as tc:
        _body(tc, goals, obss, w_go, b_go, posT_d, wqa, wqb, wka, wkb, wv_hd,
              wpa, wpb, wf1, wf2, wact_d, ln1_g, ln1_b, ln2_g, ln2_b,
              b_proj, b_ff1, b_ff2, lnf_g, lnf_b, b_act, selA_d, selB_d, out)
    return nc


def _body(tc, goals, obss, w_go, b_go, posT_d, wqa, wqb, wka, wkb, wv_hd,
          wpa, wpb, wf1, wf2, wact_d, ln1_g, ln1_b, ln2_g, ln2_b,
          b_proj, b_ff1, b_ff2, lnf_g, lnf_b, b_act, selA_d, selB_d, out):
    from contextlib import ExitStack
    nc = tc.nc
    ctx = ExitStack()

    wp = ctx.enter_context(tc.tile_pool(name="wp", bufs=1))     # weights/consts
    xp = ctx.enter_context(tc.tile_pool(name="xp", bufs=1))     # residual streams
    tp = ctx.enter_context(tc.tile_pool(name="tp", bufs=2))     # transient small
    xnp = ctx.enter_context(tc.tile_pool(name="xnp", bufs=2))   # LN outputs
    qkp = ctx.enter_context(tc.tile_pool(name="qkp", bufs=2))   # q/k spreads
    vp = ctx.enter_context(tc.tile_pool(name="vp", bufs=2))     # v tiles
    ep = ctx.enter_context(tc.tile_pool(name="ep", bufs=26))    # exp(att) blocks
    op_ = ctx.enter_context(tc.tile_pool(name="op", bufs=2))    # attn out concat
    hp = ctx.enter_context(tc.tile_pool(name="hp", bufs=2))     # ff hidden
    pp = ctx.enter_context(tc.tile_pool(name="pp", bufs=8, space="PSUM"))

    def psum(p, f, tag="ps"):
        return pp.tile([p, f], F32, tag=tag, name=tag)

    # ---------------- input prefetch ----------------
    # Issue the (small) activation-input DMAs before the ~3MB of weight DMAs
    # that share the sync queue, so the embed section starts immediately
    # instead of waiting ~75us for the whole weight stream.
    inp = ctx.enter_context(tc.tile_pool(name="inp", bufs=1))
    OT, GT = [], []
    for b in range(BPC):
        row = []
        for t in range(NT):
            otile = inp.tile([128, DO], F32, tag=f"ot{b}_{t}", name=f"ot{b}_{t}")
            nc.sync.dma_start(out=otile[:], in_=obss[b, 128 * t:128 * (t + 1), :])
            row.append(otile)
        OT.append(row)
        gT = inp.tile([DG, 1], F32, tag=f"gT{b}", name=f"gT{b}")
        nc.sync.dma_start(out=gT[:], in_=goals[b, 0, :, None])
        GT.append(gT)

    # ---------------- constants ----------------
    ident = wp.tile([128, 128], F32, tag="ident")
    make_identity(nc, ident[:])
    ones96b = wp.tile([96, 1], BF16, tag="ones96b")
    nc.vector.memset(ones96b[:], 1.0)
    ones1r = wp.tile([1, D], BF16, tag="ones1r")
    nc.vector.memset(ones1r[:], 1.0)
    eps_c = wp.tile([1, 1], F32, tag="eps_c")
    nc.vector.memset(eps_c[:], EPS)

    if MASK_TRI:
        # tri[x, y] = NEG if x < y else 0 ; idm = [I128 | 0] (bf16)
        tri = wp.tile([128, 128], BF16, tag="tri")
        nc.gpsimd.memset(tri[:], NEG)
        nc.gpsimd.affine_select(
            out=tri[:], in_=tri[:], pattern=[[-1, 128]],
            compare_op=mybir.AluOpType.is_lt, fill=0.0, base=0,
            channel_multiplier=1)
        idm = wp.tile([128, 128], BF16, tag="idm")
        make_identity(nc, idm[:])

    # b_act broadcast to [128, DA] (free-dim bias for the final head)
    bact_row = wp.tile([1, DA], F32, tag="bact_row")
    nc.sync.dma_start(out=bact_row[:], in_=b_act[None, :])
    bact_b = wp.tile([128, DA], F32, tag="bact_b")
    nc.gpsimd.partition_broadcast(bact_b[:], bact_row[:])

    posT = wp.tile([D, T], F32, tag="posT")
    nc.sync.dma_start(out=posT[:], in_=posT_d[:, :])

    wgo_g = wp.tile([DG, D], F32, tag="wgo_g")
    nc.sync.dma_start(out=wgo_g[:], in_=w_go[0:DG, :])
    wgo_o = wp.tile([DO, D], F32, tag="wgo_o")
    nc.sync.dma_start(out=wgo_o[:], in_=w_go[DG:DG + DO, :])
    bgo = wp.tile([D, 1], F32, tag="bgo")
    nc.sync.dma_start(out=bgo[:], in_=b_go[:, None])

    # per-layer weights: direct bf16 DMAs (host pre-laid-out)
    WQA, WQB, WKA, WKB, WV, WP, WF1, WF2 = [], [], [], [], [], [], [], []
    L1G, L1B, L2G, L2B, BP, BF1, BF2 = [], [], [], [], [], [], []

    def wtile(src, shape, tag, dt=BF16):
        t_ = wp.tile(list(shape), dt, tag=tag, name=tag)
        nc.sync.dma_start(out=t_[:], in_=src)
        return t_

    for l in range(L):
        WQA.append(wtile(wqa[l], [D, 112], f"wqA{l}"))
        WQB.append(wtile(wqb[l], [D, 48], f"wqB{l}"))
        WKA.append(wtile(wka[l], [D, 112], f"wkA{l}"))
        WKB.append(wtile(wkb[l], [D, 48], f"wkB{l}"))
        WV.append(wtile(wv_hd[l], [D + 1, NH * 32], f"wv{l}"))
        WP.append((wtile(wpa[l], [128, D], f"wpA{l}"),
                   wtile(wpb[l], [64, D], f"wpB{l}")))
        WF1.append(wtile(wf1[l], [D, FF], f"wf1{l}"))
        WF2.append([wtile(wf2[l, 128 * f:128 * (f + 1), :], [128, D], f"wf2{l}_{f}")
                    for f in range(3)])

        def colv(src, tag, p=D):
            t_ = wp.tile([p, 1], F32, tag=tag, name=tag)
            nc.sync.dma_start(out=t_[:], in_=src[:, None])
            return t_
        L1G.append(colv(ln1_g[l], f"l1g{l}"))
        L1B.append(colv(ln1_b[l], f"l1b{l}"))
        L2G.append(colv(ln2_g[l], f"l2g{l}"))
        L2B.append(colv(ln2_b[l], f"l2b{l}"))
        BP.append(colv(b_proj[l], f"bp{l}"))
        BF2.append(colv(b_ff2[l], f"bf2{l}"))
        BF1.append([colv(b_ff1[l, 128 * f:128 * (f + 1)], f"bf1{l}_{f}", p=128)
                    for f in range(3)])

    lnfg = wp.tile([D, 1], F32, tag="lnfg")
    nc.sync.dma_start(out=lnfg[:], in_=lnf_g[:, None])
    lnfb = wp.tile([D, 1], F32, tag="lnfb")
    nc.sync.dma_start(out=lnfb[:], in_=lnf_b[:, None])
    wact = wtile(wact_d[:, :], [D, DA], "wact")
    selA = wtile(selA_d[:, :], [128, 128], "selA")
    selB = wtile(selB_d[:, :], [64, 64], "selB")

    # ---------------- embedding ----------------
    XT = []
    obsp_cm = tc.tile_pool(name="obsp", bufs=1)
    obsp = obsp_cm.__enter__()
    for b in range(BPC):
        with nc.named_scope(f"embed{b}"):
            obsT = obsp.tile([DO, T], F32, tag=f"obsT{b}", name=f"obsT{b}")
            for t in range(NT):
                ps = psum(DO, 128)
                nc.tensor.transpose(ps[:], OT[b][t][:], ident[:])
                nc.vector.tensor_copy(out=obsT[:, 128 * t:128 * (t + 1)], in_=ps[:])
            cps = psum(D, 1)
            nc.tensor.matmul(cps[:], wgo_g[:], GT[b][:], start=True, stop=True)
            cgo = tp.tile([D, 1], F32, tag="cgo")
            nc.vector.tensor_add(out=cgo[:], in0=cps[:], in1=bgo[:])
            xT = xp.tile([D, T], F32, tag=f"xT{b}")
            for c in range(NCH):
                sl = slice(CW * c, CW * (c + 1))
                ps = psum(D, CW)
                nc.tensor.matmul(ps[:], wgo_o[:], obsT[:, sl],
                                 start=True, stop=True)
                nc.vector.tensor_scalar(out=xT[:, sl], in0=ps[:], scalar1=cgo[:],
                                        scalar2=None, op0=mybir.AluOpType.add)
                nc.vector.tensor_add(out=xT[:, sl], in0=xT[:, sl], in1=posT[:, sl])
            XT.append(xT)
    obsp_cm.__exit__(None, None, None)

    # ---------------- helpers ----------------
    def layernorm(xT, g, b, tag):
        """x [D, T] -> (x - mean)/sqrt(var+eps) * g + b, stats over D (partitions).

        Works on a bf16 copy of x: the stats matmuls stream bf16 (4x faster
        than fp32 on the PE) and the elementwise passes run at 2x DVE rate.
        The xn tile has one spare row (D) used by the caller as a constant-ones
        contraction row for the V matmul."""
        xn = xnp.tile([D + 1, T], BF16, tag=tag, name=tag)
        for c in range(NCH):
            sl = slice(CW * c, CW * (c + 1))
            xb = tp.tile([D, CW], BF16, tag="xb")
            nc.vector.tensor_copy(out=xb[:], in_=xT[:, sl])
            x2c = tp.tile([D, CW], BF16, tag="x2c")
            nc.vector.tensor_mul(out=x2c[:], in0=xb[:], in1=xb[:])
            ps1 = psum(1, CW)
            nc.tensor.matmul(ps1[:], ones96b[:], xb[:], start=True, stop=True)
            ps2 = psum(1, CW)
            nc.tensor.matmul(ps2[:], ones96b[:], x2c[:], start=True, stop=True)
            # m = s1/96 ; var = s2/96 - m^2 ; rstd = exp(-0.5*ln(var+eps))
            m_sb = tp.tile([1, CW], F32, tag="m_sb")
            nc.vector.tensor_scalar(out=m_sb[:], in0=ps1[:], scalar1=1.0 / D,
                                    scalar2=None, op0=mybir.AluOpType.mult)
            msq = tp.tile([1, CW], F32, tag="msq")
            nc.vector.tensor_mul(out=msq[:], in0=m_sb[:], in1=m_sb[:])
            var = tp.tile([1, CW], F32, tag="var")
            nc.vector.tensor_scalar(out=var[:], in0=ps2[:], scalar1=1.0 / D,
                                    scalar2=None, op0=mybir.AluOpType.mult)
            nc.vector.tensor_sub(out=var[:], in0=var[:], in1=msq[:])
            nc.scalar.activation(out=var[:], in_=var[:],
                                 func=mybir.ActivationFunctionType.Ln,
                                 bias=eps_c[:])
            rs_sb = tp.tile([1, CW], F32, tag="rs_sb")
            nc.scalar.activation(out=rs_sb[:], in_=var[:],
                                 func=mybir.ActivationFunctionType.Exp, scale=-0.5)
            m_sbh = tp.tile([1, CW], BF16, tag="m_sbh")
            nc.vector.tensor_copy(out=m_sbh[:], in_=m_sb[:])
            rs_sbh = tp.tile([1, CW], BF16, tag="rs_sbh")
            nc.vector.tensor_copy(out=rs_sbh[:], in_=rs_sb[:])
            # broadcast m/rstd across partitions with K=1 matmuls (~0.2us each
            # on the PE) instead of gpsimd partition_broadcast (~1.5us each)
            m_b = psum(D, CW)
            nc.tensor.matmul(m_b[:], ones1r[:], m_sbh[:], start=True, stop=True)
            rs_b = psum(D, CW)
            nc.tensor.matmul(rs_b[:], ones1r[:], rs_sbh[:], start=True, stop=True)
            xnf = tp.tile([D, CW], BF16, tag="xnf")
            nc.vector.tensor_sub(out=xnf[:], in0=xb[:], in1=m_b[:])
            nc.vector.tensor_mul(out=xnf[:], in0=xnf[:], in1=rs_b[:])
            nc.vector.tensor_scalar(out=xn[0:D, sl], in0=xnf[:], scalar1=g[:],
                                    scalar2=b[:], op0=mybir.AluOpType.mult,
                                    op1=mybir.AluOpType.add)
        return xn

    # ---------------- transformer layers ----------------
    # Staggered schedule: while batch element b runs its attention (PE/Scalar
    # heavy), the OTHER element's post-attention work (proj/ln2/ff) and next
    # layer's ln1/qkv (Vector heavy) are already in the engine queues, so the
    # DVE-bound and PE/ACT-bound phases of the two elements overlap instead of
    # spiking the same engine together.
    SPR = [None] * BPC
    VG = [None] * BPC
    OC = [None] * BPC
    LNOUT = {}

    def ln_gen(xT, g, bb, tag, scope):
        """generator version of layernorm: yields once per chunk; the produced
        tile is in LNOUT[tag] after exhaustion (same tile on every chunk)."""
        xn = xnp.tile([D + 1, T], BF16, tag=tag, name=tag)
        LNOUT[tag] = xn
        for c in range(NCH):
            with nc.named_scope(scope):
                sl = slice(CW * c, CW * (c + 1))
                xb = tp.tile([D, CW], BF16, tag="xb")
                nc.vector.tensor_copy(out=xb[:], in_=xT[:, sl])
                x2c = tp.tile([D, CW], BF16, tag="x2c")
                nc.vector.tensor_mul(out=x2c[:], in0=xb[:], in1=xb[:])
                ps1 = psum(1, CW)
                nc.tensor.matmul(ps1[:], ones96b[:], xb[:], start=True, stop=True)
                ps2 = psum(1, CW)
                nc.tensor.matmul(ps2[:], ones96b[:], x2c[:], start=True, stop=True)
                m_sb = tp.tile([1, CW], F32, tag="m_sb")
                nc.vector.tensor_scalar(out=m_sb[:], in0=ps1[:], scalar1=1.0 / D,
                                        scalar2=None, op0=mybir.AluOpType.mult)
                msq = tp.tile([1, CW], F32, tag="msq")
                nc.vector.tensor_mul(out=msq[:], in0=m_sb[:], in1=m_sb[:])
                var = tp.tile([1, CW], F32, tag="var")
                nc.vector.tensor_scalar(out=var[:], in0=ps2[:], scalar1=1.0 / D,
                                        scalar2=None, op0=mybir.AluOpType.mult)
                nc.vector.tensor_sub(out=var[:], in0=var[:], in1=msq[:])
                nc.scalar.activation(out=var[:], in_=var[:],
                                     func=mybir.ActivationFunctionType.Ln,
                                     bias=eps_c[:])
                rs_sb = tp.tile([1, CW], F32, tag="rs_sb")
                nc.scalar.activation(out=rs_sb[:], in_=var[:],
                                     func=mybir.ActivationFunctionType.Exp,
                                     scale=-0.5)
                m_sbh = tp.tile([1, CW], BF16, tag="m_sbh")
                nc.vector.tensor_copy(out=m_sbh[:], in_=m_sb[:])
                rs_sbh = tp.tile([1, CW], BF16, tag="rs_sbh")
                nc.vector.tensor_copy(out=rs_sbh[:], in_=rs_sb[:])
                m_b = psum(D, CW)
                nc.tensor.matmul(m_b[:], ones1r[:], m_sbh[:], start=True, stop=True)
                rs_b = psum(D, CW)
                nc.tensor.matmul(rs_b[:], ones1r[:], rs_sbh[:],
                                 start=True, stop=True)
                xnf = tp.tile([D, CW], BF16, tag="xnf")
                nc.vector.tensor_sub(out=xnf[:], in0=xb[:], in1=m_b[:])
                nc.vector.tensor_mul(out=xnf[:], in0=xnf[:], in1=rs_b[:])
                nc.vector.tensor_scalar(out=xn[0:D, sl], in0=xnf[:], scalar1=g[:],
                                        scalar2=bb[:], op0=mybir.AluOpType.mult,
                                        op1=mybir.AluOpType.add)
            yield

    def pre_gen(b, l):
        """ln1 + qkv projections for (b, l), yielding between op groups."""
        yield from ln_gen(XT[b], L1G[l], L1B[l], "xn1", f"L{l}b{b}.ln1")
        xn = LNOUT["xn1"]
        # q/k spread tiles: head h<4 at rows 32h of A, h>=4 at 32(h-4) of B
        qA = qkp.tile([112, T], BF16, tag="qA")
        qB = qkp.tile([48, T], BF16, tag="qB")
        kA = qkp.tile([112, T], BF16, tag="kA")
        kB = qkp.tile([48, T], BF16, tag="kB")
        with nc.named_scope(f"L{l}b{b}.qkv"):
            # constant-ones contraction row for the V matmul: makes WV's
            # ones-block produce 1.0 columns so each 32-wide vaug band is
            # [16 ones cols | 16 V cols] with no strided copies.
            nc.vector.memset(xn[D:D + 1, :], 1.0)
        for c in range(NCH):
            with nc.named_scope(f"L{l}b{b}.qkv"):
                sl = slice(CW * c, CW * (c + 1))
                for dst, w in ((qA, WQA[l]), (qB, WQB[l]),
                               (kA, WKA[l]), (kB, WKB[l])):
                    p = dst.shape[0]
                    ps = psum(p, CW)
                    nc.tensor.matmul(ps[:], w[:], xn[0:D, sl],
                                     start=True, stop=True)
                    nc.vector.tensor_copy(out=dst[:, sl], in_=ps[:])
            yield
        # vaug: per key-tile t, head h at cols 192t+32h: 16 all-ones cols
        # then 16 V cols, so the 32-wide AV matmul band also produces the
        # softmax denominator replicated on psum rows col..col+16
        vaug = vp.tile([128, 192 * NT], EDT, tag="vaug")
        for half in range(2):
            with nc.named_scope(f"L{l}b{b}.qkv"):
                for t in range(NT // 2 * half, NT // 2 * (half + 1)):
                    ps = psum(128, NH * 32)
                    nc.tensor.matmul(ps[:], xn[0:D + 1, 128 * t:128 * (t + 1)],
                                     WV[l][:], start=True, stop=True)
                    nc.vector.tensor_copy(out=vaug[:, 192 * t:192 * (t + 1)],
                                          in_=ps[:])
            yield
        SPR[b] = (qA, qB, kA, kB)
        VG[b] = vaug

    def att_gen(b, l):
        """full causal attention for (b, l): fills OC[b]."""
        qA, qB, kA, kB = SPR[b]
        vaug = VG[b]
        ocatA = op_.tile([128, T], BF16, tag="ocatA", name="ocatA")
        ocatB = op_.tile([64, T], BF16, tag="ocatB", name="ocatB")
        OC[b] = (ocatA, ocatB)
        for c in range(NCH):
            smax = 4 * c + 3
            oA = psum(128, CW)
            oB = psum(64, CW)
            pend = []  # (s, h, et, n0)

            def flush_av(lim, oA=oA, oB=oB, smax=smax):
                # AV matmuls trail the QK/exp stream by one whole key-step:
                # the PE never waits on exp, and QK (row-tiled) / AV
                # (col-tiled) run as same-mode bursts.
                while len(pend) > lim:
                    fs, h, et, n0 = pend.pop(0)
                    o, col = (oA, 32 * h) if h < 4 else (oB, 32 * (h - 4))
                    nc.tensor.matmul(
                        o[col:col + 32, n0:CW],
                        vaug[:, 192 * fs + 32 * h:192 * fs + 32 * h + 32],
                        et[:, n0:CW],
                        start=(fs == 0), stop=(fs == smax),
                        tile_position=(0, col), skip_group_check=True)

            for s in range(smax + 1):
                crossed = s >= 4 * c
                n0 = 128 * s - CW * c if crossed else 0
                with nc.named_scope(f"L{l}b{b}.att{c}"):
                    for h in range(NH):
                        spr_k, spr_q, row = ((kA, qA, 32 * h) if h < 4
                                             else (kB, qB, 32 * (h - 4)))
                        pqk = psum(128, CW)
                        nc.tensor.matmul(
                            pqk[:, 0:CW - n0],
                            spr_k[row:row + HS, 128 * s:128 * (s + 1)],
                            spr_q[row:row + HS, CW * c + n0:CW * (c + 1)],
                            start=True, stop=not (crossed and MASK_TRI),
                            tile_position=(row, 0),
                            skip_group_check=True)
                        if crossed and MASK_TRI:
                            nc.tensor.matmul(
                                pqk[:, 0:128], tri[:], idm[:],
                                start=False, stop=True,
                                tile_position=(0, 0), skip_group_check=True)
                        et = ep.tile([128, CW], EDT, tag="et")
                        if crossed and not MASK_TRI:
                            # diagonal band: exp into a scratch tile, then
                            # causal-select into et; rest exps directly
                            if CW - n0 > 128:
                                nc.scalar.activation(
                                    out=et[:, n0 + 128:CW],
                                    in_=pqk[:, 128:CW - n0],
                                    func=mybir.ActivationFunctionType.Exp)
                            etb = ep.tile([128, 128], EDT, tag="etb", bufs=12)
                            nc.scalar.activation(
                                out=etb[:], in_=pqk[:, 0:128],
                                func=mybir.ActivationFunctionType.Exp)
                            nc.gpsimd.affine_select(
                                out=et[:, n0:n0 + 128], in_=etb[:],
                                pattern=[[1, 128]],
                                compare_op=mybir.AluOpType.is_ge,
                                fill=0.0, base=0, channel_multiplier=-1)
                        else:
                            nc.scalar.activation(
                                out=et[:, n0:CW], in_=pqk[:, 0:CW - n0],
                                func=mybir.ActivationFunctionType.Exp)
                        pend.append((s, h, et, n0))
                    flush_av(NH if s < smax else 0)
                yield
            with nc.named_scope(f"L{l}b{b}.attep{c}"):
                # softmax denominators sit (replicated) on psum rows
                # col..col+16 of each band. Broadcast across the full 32-row
                # band with a constant selection matmul (sel[k,p]=1 iff
                # k == 32*(p//32)) via an SBUF bounce copy, reciprocal, then
                # scale. Rows col..col+16 of each ocat band become
                # sigma/sigma = 1.0 and hit zero rows of w_proj downstream.
                sden = tp.tile([128, CW], BF16, tag="sden")
                sdenB = tp.tile([64, CW], BF16, tag="sdenB")
                nc.vector.tensor_copy(out=sden[:], in_=oA[:])
                nc.vector.tensor_copy(out=sdenB[:], in_=oB[:])
                sbA = psum(128, CW)
                nc.tensor.matmul(sbA[:], selA[:], sden[:], start=True, stop=True)
                sbB = psum(64, CW)
                nc.tensor.matmul(sbB[:], selB[:], sdenB[:], start=True, stop=True)
                rbr = tp.tile([128, CW], F32, tag="rbr")
                rbrB = tp.tile([64, CW], F32, tag="rbrB")
                nc.vector.reciprocal_approx_fast(out=rbr[:], in_=sbA[:])
                nc.vector.reciprocal_approx_fast(out=rbrB[:], in_=sbB[:])
                for h in range(NH):
                    o, col = (oA, 32 * h) if h < 4 else (oB, 32 * (h - 4))
                    oc = ocatA if h < 4 else ocatB
                    rt = rbr if h < 4 else rbrB
                    nc.vector.tensor_mul(
                        out=oc[col:col + 32, CW * c:CW * (c + 1)],
                        in0=o[col:col + 32, :], in1=rt[col:col + 32, :])
            yield

    def post_gen(b, l):
        """proj + ln2 + ff for (b, l), yielding between op groups."""
        xT = XT[b]
        ocatA, ocatB = OC[b]
        for c in range(NCH):
            with nc.named_scope(f"L{l}b{b}.proj"):
                sl = slice(CW * c, CW * (c + 1))
                ps = psum(D, CW)
                nc.tensor.matmul(ps[:], WP[l][0][:], ocatA[:, sl],
                                 start=True, stop=False)
                nc.tensor.matmul(ps[:], WP[l][1][:], ocatB[:, sl],
                                 start=False, stop=True)
                nc.vector.tensor_scalar(out=ps[:], in0=ps[:], scalar1=BP[l][:],
                                        scalar2=None, op0=mybir.AluOpType.add)
                nc.vector.tensor_add(out=xT[:, sl], in0=xT[:, sl], in1=ps[:])
            yield
        yield from ln_gen(xT, L2G[l], L2B[l], "xn2", f"L{l}b{b}.ln2")
        xn2 = LNOUT["xn2"]
        h1 = [hp.tile([128, T], BF16, tag=f"h1_{f}", name=f"h1_{f}")
              for f in range(3)]
        for c in range(NCH):
            with nc.named_scope(f"L{l}b{b}.ff"):
                sl = slice(CW * c, CW * (c + 1))
                for f in range(3):
                    ps = psum(128, CW)
                    nc.tensor.matmul(ps[:], WF1[l][:, 128 * f:128 * (f + 1)],
                                     xn2[0:D, sl], start=True, stop=True)
                    nc.vector.tensor_scalar(
                        out=h1[f][:, sl], in0=ps[:], scalar1=BF1[l][f][:],
                        scalar2=0.0, op0=mybir.AluOpType.add,
                        op1=mybir.AluOpType.max)
            yield
        for c in range(NCH):
            with nc.named_scope(f"L{l}b{b}.ff"):
                sl = slice(CW * c, CW * (c + 1))
                ps = psum(D, CW)
                for f in range(3):
                    nc.tensor.matmul(ps[:], WF2[l][f][:], h1[f][:, sl],
                                     start=(f == 0), stop=(f == 2))
                nc.vector.tensor_scalar(out=ps[:], in0=ps[:], scalar1=BF2[l][:],
                                        scalar2=None, op0=mybir.AluOpType.add)
                nc.vector.tensor_add(out=xT[:, sl], in0=xT[:, sl], in1=ps[:])
            yield

    def head_gen(b):
        yield from ln_gen(XT[b], lnfg, lnfb, "xn1", f"head{b}")
        xnf = LNOUT["xn1"]
        for half in range(2):
            with nc.named_scope(f"head{b}"):
                for t in range(NT // 2 * half, NT // 2 * (half + 1)):
                    ps = psum(128, DA)
                    nc.tensor.matmul(ps[:], xnf[0:D, 128 * t:128 * (t + 1)],
                                     wact[:], start=True, stop=True)
                    osb = tp.tile([128, DA], F32, tag="osb")
                    nc.vector.tensor_add(out=osb[:], in0=ps[:], in1=bact_b[:])
                    nc.sync.dma_start(out=out[b, 128 * t:128 * (t + 1), :],
                                      in_=osb[:])
            yield

    def chain(*gens):
        for g in gens:
            yield from g

    def drain(g):
        for _ in g:
            pass

    def zip_emit(a, bgen):
        """Interleave two emission streams so the per-engine FIFOs see their
        ops finely interleaved (real software pipelining across the two batch
        elements)."""
        while True:
            sa = next(a, _STOP)
            sb = next(bgen, _STOP)
            if sa is _STOP and sb is _STOP:
                return

    _STOP = object()

    # Software-pipelined driver: while att(b) streams on PE/Scalar, the other
    # element's proj/ln2/ff and next-layer ln1/qkv (Vector-heavy) emit
    # interleaved, so every engine queue always holds ready work.
    drain(pre_gen(0, 0))
    carry = pre_gen(1, 0)  # b1 work that overlaps b0's attention
    for l in range(L):
        zip_emit(att_gen(0, l), carry)
        tail0 = pre_gen(0, l + 1) if l + 1 < L else head_gen(0)
        zip_emit(att_gen(1, l), chain(post_gen(0, l), tail0))
        carry = chain(post_gen(1, l),
                      pre_gen(1, l + 1) if l + 1 < L else head_gen(1))
    drain(carry)

    ctx.close()


_CACHED = None


def _pin_act_tables():
    # Pin every ACT function to the one set containing both Ln and Exp so the
    # table is loaded once instead of thrashing between sets per activation.
    from concourse import hw_specs
    import concourse.bacc as bacc_mod
    if getattr(hw_specs, "_act_tables_pinned", False):
        return
    orig = hw_specs.get_activation_tables

    def pinned(arch):
        t = orig(arch)
        keep = "natural_log_exp_and_others"
        return {n: (f if n == keep else set()) for n, f in t.items()}

    hw_specs.get_activation_tables = pinned
    if hasattr(bacc_mod, "get_activation_tables"):
        bacc_mod.get_activation_tables = pinned
    hw_specs._act_tables_pinned = True


def _get_nc():
    global _CACHED
    if _CACHED is None:
        _pin_act_tables()
        nc = bacc.Bacc("TRN2", target_bir_lowering=False, debug=False,
                       enable_asserts=False)
        build(nc)
        nc.compile()
        _CACHED = nc
    return _CACHED


def prep_weights(inputs):
    """Host-side layout/casting of the (replicated) weights."""
    f = {k: _dt(v) for k, v in inputs.items()}
    o = {}
    o["w_go"] = f["w_go"]
    o["b_go"] = f["b_go"]
    o["posT"] = np.ascontiguousarray(f["pos_emb"].T)
    wq, wk, wv, wpj = f["wq"], f["wk"], f["wv"], f["w_proj"]
    qa = np.zeros((L, D, 112), np.float32)
    qb = np.zeros((L, D, 48), np.float32)
    ka = np.zeros((L, D, 112), np.float32)
    kb = np.zeros((L, D, 48), np.float32)
    pa = np.zeros((L, 128, D), np.float32)
    pb = np.zeros((L, 64, D), np.float32)
    for h in range(NH):
        if h < 4:
            qa[:, :, 32 * h:32 * h + HS] = wq[:, h] * SCALE
            ka[:, :, 32 * h:32 * h + HS] = wk[:, h]
            # ocat rows col..col+16 are sigma/sigma = 1.0; o sits at +16
            pa[:, 32 * h + HS:32 * h + 32, :] = wpj[:, HS * h:HS * (h + 1), :]
        else:
            r = 32 * (h - 4)
            qb[:, :, r:r + HS] = wq[:, h] * SCALE
            kb[:, :, r:r + HS] = wk[:, h]
            pb[:, r + HS:r + 32, :] = wpj[:, HS * h:HS * (h + 1), :]
    o["wqa"] = qa.astype(NPBF)
    o["wqb"] = qb.astype(NPBF)
    o["wka"] = ka.astype(NPBF)
    o["wkb"] = kb.astype(NPBF)
    # wv_hd [L, D+1, NH*32]: per head, cols [32h:32h+16) come from the
    # constant-ones xn row (row D) -> all-ones output; cols [32h+16:32h+32)
    # are the V projection.
    wvp = np.zeros((L, D + 1, NH * 32), np.float32)
    for h in range(NH):
        wvp[:, 0:D, 32 * h + HS:32 * h + 32] = wv[:, h]
        wvp[:, D, 32 * h:32 * h + HS] = 1.0
    o["wv_hd"] = wvp.astype(NPBF)
    o["wpa"] = pa.astype(NPBF)
    o["wpb"] = pb.astype(NPBF)
    o["wf1"] = f["w_ff1"].astype(NPBF)
    o["wf2"] = f["w_ff2"].astype(NPBF)
    o["wact"] = f["w_act"].astype(NPBF)
    sa = np.zeros((128, 128), np.float32)
    sa[32 * (np.arange(128) // 32), np.arange(128)] = 1.0
    o["selA"] = sa.astype(NPBF)
    o["selB"] = np.ascontiguousarray(sa[:64, :64]).astype(NPBF)
    for k in ("ln1_g", "ln1_b", "ln2_g", "ln2_b", "b_proj", "b_ff1", "b_ff2",
              "lnf_g", "lnf_b", "b_act"):
        o[k] = f[k]
    return o, f


def kernel(**inputs):
    global LAST_RESULT
    nc = _get_nc()
    w, f = prep_weights(inputs)
    in_maps = []
    for i in range(NCORES):
        m = dict(w)
        m["goals"] = f["goals"][BPC * i:BPC * (i + 1)]
        m["obss"] = f["obss"][BPC * i:BPC * (i + 1)]
        in_maps.append(m)
    res = run_bass_kernel_spmd(nc, in_maps, core_ids=list(range(NCORES)))
    LAST_RESULT = res
    return np.concatenate([r["out"] for r in res.results], axis=0)


# revision 53
# speedup vs baseline: 1.4342x; 1.0594x over previous
"""Trainium2 Bass kernel for nn_ActorTransformer (B=16, T=1024, D=96, L=6, NH=6).

Strategy: pure data parallelism, 2 batch elements per NeuronCore on 8 cores.
Inside a core, activations live transposed as [D=96 partitions, T=1024 free].
Attention is computed as attT [Tk, Tq] blocks so the AV matmul needs no
transposes; per-head matmuls are packed into 32-row/col PE tile groups and
run in bf16.

v2 changes vs baseline:
- softmax denominators are folded into the AV matmul as an extra all-ones
  column per head (V bands are 17 wide), killing the dedicated ones-lhsT
  denominator matmuls (~190us of PE time).
- causal trimming: for diagonal-crossing key tiles, QK/exp/AV only cover the
  q-column range that has any unmasked key (saves ~25% of attention work).
- the causal mask is applied only to the 128-col crossing band (in-place
  affine_select), or optionally as a triangle-constant accumulating matmul
  (MASK_TRI=1).
- scale 1/sqrt(hs) folded into the host-side Wq layout.
- per-iteration memsets (ocat, dP) removed; constant regions are written once.
"""

import os
import sys

import ml_dtypes
import numpy as np

sys.path.insert(0, "/opt/trn_rl_repo")

import concourse.bass as bass
import concourse.mybir as mybir
import concourse.tile as tile
from concourse import bacc
from concourse.bass_utils import run_bass_kernel_spmd
from concourse.masks import make_identity

# Model constants
L, NH, D, HS, FF = 6, 6, 96, 16, 384
DG, DO, DA = 32, 64, 16
T, B = 1024, 16
NCORES = 8
BPC = B // NCORES  # batch per core
SCALE = HS ** -0.5  # 0.25
EPS = 1e-5
CW = 512  # chunk width (matmul moving free dim)
NCH = T // CW  # 2
NT = T // 128  # 8 token tiles
F32 = mybir.dt.float32
BF16 = mybir.dt.bfloat16
NPBF = ml_dtypes.bfloat16
ATT_F32 = os.environ.get("ATT_F32", "0") == "1"
EDT = F32 if ATT_F32 else BF16
MASK_TRI = os.environ.get("MASK_TRI", "0") == "1"
NEG = -60000.0
# Schraudolph exp on gpsimd for heads 4/5: exp(s) ~ bitcast(int32(a*s + b))
# with a = 2^23*log2(e) folded into the host-side Wq of those heads.
SCHRAUD = os.environ.get("SCHRAUD", "0") == "1"
SCH_A = 12102203.161561485  # 2^23 / ln 2
SCH_B = float(127 * (1 << 23) - 486411)  # bias - Schraudolph C
I32 = mybir.dt.int32

LAST_RESULT = None


def _dt(x):
    return np.ascontiguousarray(np.asarray(x, dtype=np.float32))


def build(nc):
    """Builds the full forward pass for BPC batch elements on one core."""
    def din(name, shape, dt=F32):
        return nc.dram_tensor(name, list(shape), dt, kind="ExternalInput").ap()

    goals = din("goals", [BPC, 1, DG])
    obss = din("obss", [BPC, T, DO])
    w_go = din("w_go", [DG + DO, D])
    b_go = din("b_go", [D])
    posT_d = din("posT", [D, T])
    wqa = din("wqa", [L, D, 112], BF16)
    wqb = din("wqb", [L, D, 48], BF16)
    wka = din("wka", [L, D, 112], BF16)
    wkb = din("wkb", [L, D, 48], BF16)
    wv_hd = din("wv_hd", [L, D + 1, NH * 32], BF16)
    wpa = din("wpa", [L, 128, D], BF16)
    wpb = din("wpb", [L, 64, D], BF16)
    wf1 = din("wf1", [L, D, FF], BF16)
    wf2 = din("wf2", [L, FF, D], BF16)
    wact_d = din("wact", [D, DA], BF16)
    ln1_g = din("ln1_g", [L, D])
    ln1_b = din("ln1_b", [L, D])
    ln2_g = din("ln2_g", [L, D])
    ln2_b = din("ln2_b", [L, D])
    b_proj = din("b_proj", [L, D])
    b_ff1 = din("b_ff1", [L, FF])
    b_ff2 = din("b_ff2", [L, D])
    lnf_g = din("lnf_g", [D])
    lnf_b = din("lnf_b", [D])
    b_act = din("b_act", [DA])
    selA_d = din("selA", [128, 128], BF16)
    selB_d = din("selB", [64, 64], BF16)
    out = nc.dram_tensor("out", [BPC, T, DA], F32, kind="ExternalOutput").ap()

    with tile.TileContext(nc) as tc:
        _body(tc, goals, obss, w_go, b_go, posT_d, wqa, wqb, wka, wkb, wv_hd,
              wpa, wpb, wf1, wf2, wact_d, ln1_g, ln1_b, ln2_g, ln2_b,
              b_proj, b_ff1, b_ff2, lnf_g, lnf_b, b_act, selA_d, selB_d, out)
    return nc


def _body(tc, goals, obss, w_go, b_go, posT_d, wqa, wqb, wka, wkb, wv_hd,
          wpa, wpb, wf1, wf2, wact_d, ln1_g, ln1_b, ln2_g, ln2_b,
          b_proj, b_ff1, b_ff2, lnf_g, lnf_b, b_act, selA_d, selB_d, out):
    from contextlib import ExitStack
    nc = tc.nc
    ctx = ExitStack()

    wp = ctx.enter_context(tc.tile_pool(name="wp", bufs=1))     # weights/consts
    xp = ctx.enter_context(tc.tile_pool(name="xp", bufs=1))     # residual streams
    tp = ctx.enter_context(tc.tile_pool(name="tp", bufs=2))     # transient small
    xnp = ctx.enter_context(tc.tile_pool(name="xnp", bufs=2))   # LN outputs
    qkp = ctx.enter_context(tc.tile_pool(name="qkp", bufs=2))   # q/k spreads
    vp = ctx.enter_context(tc.tile_pool(name="vp", bufs=2))     # v tiles
    ep = ctx.enter_context(tc.tile_pool(name="ep", bufs=26))    # exp(att) blocks
    op_ = ctx.enter_context(tc.tile_pool(name="op", bufs=2))    # attn out concat
    hp = ctx.enter_context(tc.tile_pool(name="hp", bufs=2))     # ff hidden
    pp = ctx.enter_context(tc.tile_pool(name="pp", bufs=8, space="PSUM"))

    def psum(p, f, tag="ps"):
        return pp.tile([p, f], F32, tag=tag, name=tag)

    # ---------------- input prefetch ----------------
    # Issue the (small) activation-input DMAs before the ~3MB of weight DMAs
    # that share the sync queue, so the embed section starts immediately
    # instead of waiting ~75us for the whole weight stream.
    inp = ctx.enter_context(tc.tile_pool(name="inp", bufs=1))
    OT, GT = [], []
    for b in range(BPC):
        row = []
        for t in range(NT):
            otile = inp.tile([128, DO], F32, tag=f"ot{b}_{t}", name=f"ot{b}_{t}")
            nc.sync.dma_start(out=otile[:], in_=obss[b, 128 * t:128 * (t + 1), :])
            row.append(otile)
        OT.append(row)
        gT = inp.tile([DG, 1], F32, tag=f"gT{b}", name=f"gT{b}")
        nc.sync.dma_start(out=gT[:], in_=goals[b, 0, :, None])
        GT.append(gT)

    # ---------------- constants ----------------
    ident = wp.tile([128, 128], F32, tag="ident")
    make_identity(nc, ident[:])
    ones96b = wp.tile([96, 1], BF16, tag="ones96b")
    nc.vector.memset(ones96b[:], 1.0)
    ones1r = wp.tile([1, D], BF16, tag="ones1r")
    nc.vector.memset(ones1r[:], 1.0)
    eps_c = wp.tile([1, 1], F32, tag="eps_c")
    nc.vector.memset(eps_c[:], EPS)

    if MASK_TRI:
        # tri[x, y] = NEG if x < y else 0 ; idm = [I128 | 0] (bf16)
        tri = wp.tile([128, 128], BF16, tag="tri")
        nc.gpsimd.memset(tri[:], NEG)
        nc.gpsimd.affine_select(
            out=tri[:], in_=tri[:], pattern=[[-1, 128]],
            compare_op=mybir.AluOpType.is_lt, fill=0.0, base=0,
            channel_multiplier=1)
        idm = wp.tile([128, 128], BF16, tag="idm")
        make_identity(nc, idm[:])

    # b_act broadcast to [128, DA] (free-dim bias for the final head)
    bact_row = wp.tile([1, DA], F32, tag="bact_row")
    nc.sync.dma_start(out=bact_row[:], in_=b_act[None, :])
    bact_b = wp.tile([128, DA], F32, tag="bact_b")
    nc.gpsimd.partition_broadcast(bact_b[:], bact_row[:])

    posT = wp.tile([D, T], F32, tag="posT")
    nc.sync.dma_start(out=posT[:], in_=posT_d[:, :])

    wgo_g = wp.tile([DG, D], F32, tag="wgo_g")
    nc.sync.dma_start(out=wgo_g[:], in_=w_go[0:DG, :])
    wgo_o = wp.tile([DO, D], F32, tag="wgo_o")
    nc.sync.dma_start(out=wgo_o[:], in_=w_go[DG:DG + DO, :])
    bgo = wp.tile([D, 1], F32, tag="bgo")
    nc.sync.dma_start(out=bgo[:], in_=b_go[:, None])

    # per-layer weights: direct bf16 DMAs (host pre-laid-out)
    WQA, WQB, WKA, WKB, WV, WP, WF1, WF2 = [], [], [], [], [], [], [], []
    L1G, L1B, L2G, L2B, BP, BF1, BF2 = [], [], [], [], [], [], []

    def wtile(src, shape, tag, dt=BF16):
        t_ = wp.tile(list(shape), dt, tag=tag, name=tag)
        nc.sync.dma_start(out=t_[:], in_=src)
        return t_

    for l in range(L):
        WQA.append(wtile(wqa[l], [D, 112], f"wqA{l}"))
        WQB.append(wtile(wqb[l], [D, 48], f"wqB{l}"))
        WKA.append(wtile(wka[l], [D, 112], f"wkA{l}"))
        WKB.append(wtile(wkb[l], [D, 48], f"wkB{l}"))
        WV.append(wtile(wv_hd[l], [D + 1, NH * 32], f"wv{l}"))
        WP.append((wtile(wpa[l], [128, D], f"wpA{l}"),
                   wtile(wpb[l], [64, D], f"wpB{l}")))
        WF1.append(wtile(wf1[l], [D, FF], f"wf1{l}"))
        WF2.append([wtile(wf2[l, 128 * f:128 * (f + 1), :], [128, D], f"wf2{l}_{f}")
                    for f in range(3)])

        def colv(src, tag, p=D):
            t_ = wp.tile([p, 1], F32, tag=tag, name=tag)
            nc.sync.dma_start(out=t_[:], in_=src[:, None])
            return t_
        L1G.append(colv(ln1_g[l], f"l1g{l}"))
        L1B.append(colv(ln1_b[l], f"l1b{l}"))
        L2G.append(colv(ln2_g[l], f"l2g{l}"))
        L2B.append(colv(ln2_b[l], f"l2b{l}"))
        BP.append(colv(b_proj[l], f"bp{l}"))
        BF2.append(colv(b_ff2[l], f"bf2{l}"))
        BF1.append([colv(b_ff1[l, 128 * f:128 * (f + 1)], f"bf1{l}_{f}", p=128)
                    for f in range(3)])

    lnfg = wp.tile([D, 1], F32, tag="lnfg")
    nc.sync.dma_start(out=lnfg[:], in_=lnf_g[:, None])
    lnfb = wp.tile([D, 1], F32, tag="lnfb")
    nc.sync.dma_start(out=lnfb[:], in_=lnf_b[:, None])
    wact = wtile(wact_d[:, :], [D, DA], "wact")
    selA = wtile(selA_d[:, :], [128, 128], "selA")
    selB = wtile(selB_d[:, :], [64, 64], "selB")

    # ---------------- embedding ----------------
    XT = []
    obsp_cm = tc.tile_pool(name="obsp", bufs=1)
    obsp = obsp_cm.__enter__()
    for b in range(BPC):
        with nc.named_scope(f"embed{b}"):
            obsT = obsp.tile([DO, T], F32, tag=f"obsT{b}", name=f"obsT{b}")
            for t in range(NT):
                ps = psum(DO, 128)
                nc.tensor.transpose(ps[:], OT[b][t][:], ident[:])
                nc.vector.tensor_copy(out=obsT[:, 128 * t:128 * (t + 1)], in_=ps[:])
            cps = psum(D, 1)
            nc.tensor.matmul(cps[:], wgo_g[:], GT[b][:], start=True, stop=True)
            cgo = tp.tile([D, 1], F32, tag="cgo")
            nc.vector.tensor_add(out=cgo[:], in0=cps[:], in1=bgo[:])
            xT = xp.tile([D, T], F32, tag=f"xT{b}")
            for c in range(NCH):
                sl = slice(CW * c, CW * (c + 1))
                ps = psum(D, CW)
                nc.tensor.matmul(ps[:], wgo_o[:], obsT[:, sl],
                                 start=True, stop=True)
                nc.vector.tensor_scalar(out=xT[:, sl], in0=ps[:], scalar1=cgo[:],
                                        scalar2=None, op0=mybir.AluOpType.add)
                nc.vector.tensor_add(out=xT[:, sl], in0=xT[:, sl], in1=posT[:, sl])
            XT.append(xT)
    obsp_cm.__exit__(None, None, None)

    # ---------------- helpers ----------------
    def layernorm(xT, g, b, tag):
        """x [D, T] -> (x - mean)/sqrt(var+eps) * g + b, stats over D (partitions).

        Works on a bf16 copy of x: the stats matmuls stream bf16 (4x faster
        than fp32 on the PE) and the elementwise passes run at 2x DVE rate.
        The xn tile has one spare row (D) used by the caller as a constant-ones
        contraction row for the V matmul."""
        xn = xnp.tile([D + 1, T], BF16, tag=tag, name=tag)
        for c in range(NCH):
            sl = slice(CW * c, CW * (c + 1))
            xb = tp.tile([D, CW], BF16, tag="xb")
            nc.vector.tensor_copy(out=xb[:], in_=xT[:, sl])
            x2c = tp.tile([D, CW], BF16, tag="x2c")
            nc.vector.tensor_mul(out=x2c[:], in0=xb[:], in1=xb[:])
            ps1 = psum(1, CW)
            nc.tensor.matmul(ps1[:], ones96b[:], xb[:], start=True, stop=True)
            ps2 = psum(1, CW)
            nc.tensor.matmul(ps2[:], ones96b[:], x2c[:], start=True, stop=True)
            # m = s1/96 ; var = s2/96 - m^2 ; rstd = exp(-0.5*ln(var+eps))
            m_sb = tp.tile([1, CW], F32, tag="m_sb")
            nc.vector.tensor_scalar(out=m_sb[:], in0=ps1[:], scalar1=1.0 / D,
                                    scalar2=None, op0=mybir.AluOpType.mult)
            msq = tp.tile([1, CW], F32, tag="msq")
            nc.vector.tensor_mul(out=msq[:], in0=m_sb[:], in1=m_sb[:])
            var = tp.tile([1, CW], F32, tag="var")
            nc.vector.tensor_scalar(out=var[:], in0=ps2[:], scalar1=1.0 / D,
                                    scalar2=None, op0=mybir.AluOpType.mult)
            nc.vector.tensor_sub(out=var[:], in0=var[:], in1=msq[:])
            nc.scalar.activation(out=var[:], in_=var[:],
                                 func=mybir.ActivationFunctionType.Ln,
                                 bias=eps_c[:])
            rs_sb = tp.tile([1, CW], F32, tag="rs_sb")
            nc.scalar.activation(out=rs_sb[:], in_=var[:],
                                 func=mybir.ActivationFunctionType.Exp, scale=-0.5)
            m_sbh = tp.tile([1, CW], BF16, tag="m_sbh")
            nc.vector.tensor_copy(out=m_sbh[:], in_=m_sb[:])
            rs_sbh = tp.tile([1, CW], BF16, tag="rs_sbh")
            nc.vector.tensor_copy(out=rs_sbh[:], in_=rs_sb[:])
            # broadcast m/rstd across partitions with K=1 matmuls (~0.2us each
            # on the PE) instead of gpsimd partition_broadcast (~1.5us each)
            m_b = psum(D, CW)
            nc.tensor.matmul(m_b[:], ones1r[:], m_sbh[:], start=True, stop=True)
            rs_b = psum(D, CW)
            nc.tensor.matmul(rs_b[:], ones1r[:], rs_sbh[:], start=True, stop=True)
            xnf = tp.tile([D, CW], BF16, tag="xnf")
            nc.vector.tensor_sub(out=xnf[:], in0=xb[:], in1=m_b[:])
            nc.vector.tensor_mul(out=xnf[:], in0=xnf[:], in1=rs_b[:])
            nc.vector.tensor_scalar(out=xn[0:D, sl], in0=xnf[:], scalar1=g[:],
                                    scalar2=b[:], op0=mybir.AluOpType.mult,
                                    op1=mybir.AluOpType.add)
        return xn

    # ---------------- transformer layers ----------------
    # Staggered schedule: while batch element b runs its attention (PE/Scalar
    # heavy), the OTHER element's post-attention work (proj/ln2/ff) and next
    # layer's ln1/qkv (Vector heavy) are already in the engine queues, so the
    # DVE-bound and PE/ACT-bound phases of the two elements overlap instead of
    # spiking the same engine together.
    SPR = [None] * BPC
    VG = [None] * BPC
    OC = [None] * BPC
    LNOUT = {}

    def ln_gen(xT, g, bb, tag, scope):
        """generator version of layernorm: yields once per chunk; the produced
        tile is in LNOUT[tag] after exhaustion (same tile on every chunk)."""
        xn = xnp.tile([D + 1, T], BF16, tag=tag, name=tag)
        LNOUT[tag] = xn
        for c in range(NCH):
            with nc.named_scope(scope):
                sl = slice(CW * c, CW * (c + 1))
                xb = tp.tile([D, CW], BF16, tag="xb")
                nc.vector.tensor_copy(out=xb[:], in_=xT[:, sl])
                x2c = tp.tile([D, CW], BF16, tag="x2c")
                nc.vector.tensor_mul(out=x2c[:], in0=xb[:], in1=xb[:])
                ps1 = psum(1, CW)
                nc.tensor.matmul(ps1[:], ones96b[:], xb[:], start=True, stop=True)
                ps2 = psum(1, CW)
                nc.tensor.matmul(ps2[:], ones96b[:], x2c[:], start=True, stop=True)
                m_sb = tp.tile([1, CW], F32, tag="m_sb")
                nc.vector.tensor_scalar(out=m_sb[:], in0=ps1[:], scalar1=1.0 / D,
                                        scalar2=None, op0=mybir.AluOpType.mult)
                msq = tp.tile([1, CW], F32, tag="msq")
                nc.vector.tensor_mul(out=msq[:], in0=m_sb[:], in1=m_sb[:])
                var = tp.tile([1, CW], F32, tag="var")
                nc.vector.tensor_scalar(out=var[:], in0=ps2[:], scalar1=1.0 / D,
                                        scalar2=None, op0=mybir.AluOpType.mult)
                nc.vector.tensor_sub(out=var[:], in0=var[:], in1=msq[:])
                nc.scalar.activation(out=var[:], in_=var[:],
                                     func=mybir.ActivationFunctionType.Ln,
                                     bias=eps_c[:])
                rs_sb = tp.tile([1, CW], F32, tag="rs_sb")
                nc.scalar.activation(out=rs_sb[:], in_=var[:],
                                     func=mybir.ActivationFunctionType.Exp,
                                     scale=-0.5)
                m_sbh = tp.tile([1, CW], BF16, tag="m_sbh")
                nc.vector.tensor_copy(out=m_sbh[:], in_=m_sb[:])
                rs_sbh = tp.tile([1, CW], BF16, tag="rs_sbh")
                nc.vector.tensor_copy(out=rs_sbh[:], in_=rs_sb[:])
                # broadcast m/rstd across partitions on the (mostly idle)
                # gpsimd engine; source rows sit at partition 0 (the only base
                # partition_broadcast handles correctly on HW)
                m_b = tp.tile([D, CW], BF16, tag="m_b")
                rs_b = tp.tile([D, CW], BF16, tag="rs_b")
                nc.gpsimd.partition_broadcast(m_b[:], m_sbh[:])
                nc.gpsimd.partition_broadcast(rs_b[:], rs_sbh[:])
                xnf = tp.tile([D, CW], BF16, tag="xnf")
                nc.vector.tensor_sub(out=xnf[:], in0=xb[:], in1=m_b[:])
                nc.vector.tensor_mul(out=xnf[:], in0=xnf[:], in1=rs_b[:])
                nc.vector.tensor_scalar(out=xn[0:D, sl], in0=xnf[:], scalar1=g[:],
                                        scalar2=bb[:], op0=mybir.AluOpType.mult,
                                        op1=mybir.AluOpType.add)
            yield

    def pre_gen(b, l):
        """ln1 + qkv projections for (b, l), yielding between op groups."""
        yield from ln_gen(XT[b], L1G[l], L1B[l], "xn1", f"L{l}b{b}.ln1")
        xn = LNOUT["xn1"]
        # q/k spread tiles: head h<4 at rows 32h of A, h>=4 at 32(h-4) of B
        qA = qkp.tile([112, T], BF16, tag="qA")
        qB = qkp.tile([48, T], BF16, tag="qB")
        kA = qkp.tile([112, T], BF16, tag="kA")
        kB = qkp.tile([48, T], BF16, tag="kB")
        with nc.named_scope(f"L{l}b{b}.qkv"):
            # constant-ones contraction row for the V matmul: makes WV's
            # ones-block produce 1.0 columns so each 32-wide vaug band is
            # [16 ones cols | 16 V cols] with no strided copies.
            nc.vector.memset(xn[D:D + 1, :], 1.0)
        for c in range(NCH):
            with nc.named_scope(f"L{l}b{b}.qkv"):
                sl = slice(CW * c, CW * (c + 1))
                for dst, w in ((qA, WQA[l]), (qB, WQB[l]),
                               (kA, WKA[l]), (kB, WKB[l])):
                    p = dst.shape[0]
                    ps = psum(p, CW)
                    nc.tensor.matmul(ps[:], w[:], xn[0:D, sl],
                                     start=True, stop=True)
                    nc.vector.tensor_copy(out=dst[:, sl], in_=ps[:])
            yield
        # vaug: per key-tile t, head h at cols 192t+32h: 16 all-ones cols
        # then 16 V cols, so the 32-wide AV matmul band also produces the
        # softmax denominator replicated on psum rows col..col+16
        vaug = vp.tile([128, 192 * NT], EDT, tag="vaug")
        for half in range(2):
            with nc.named_scope(f"L{l}b{b}.qkv"):
                for t in range(NT // 2 * half, NT // 2 * (half + 1)):
                    ps = psum(128, NH * 32)
                    nc.tensor.matmul(ps[:], xn[0:D + 1, 128 * t:128 * (t + 1)],
                                     WV[l][:], start=True, stop=True)
                    nc.vector.tensor_copy(out=vaug[:, 192 * t:192 * (t + 1)],
                                          in_=ps[:])
            yield
        SPR[b] = (qA, qB, kA, kB)
        VG[b] = vaug

    def att_gen(b, l):
        """full causal attention for (b, l): fills OC[b]."""
        qA, qB, kA, kB = SPR[b]
        vaug = VG[b]
        ocatA = op_.tile([128, T], BF16, tag="ocatA", name="ocatA")
        ocatB = op_.tile([64, T], BF16, tag="ocatB", name="ocatB")
        OC[b] = (ocatA, ocatB)
        for c in range(NCH):
            smax = 4 * c + 3
            oA = psum(128, CW)
            oB = psum(64, CW)
            pend = []  # (s, h, et, n0)

            def flush_av(lim, oA=oA, oB=oB, smax=smax):
                # AV matmuls trail the QK/exp stream by one whole key-step:
                # the PE never waits on exp, and QK (row-tiled) / AV
                # (col-tiled) run as same-mode bursts.
                while len(pend) > lim:
                    fs, h, et, n0 = pend.pop(0)
                    o, col = (oA, 32 * h) if h < 4 else (oB, 32 * (h - 4))
                    nc.tensor.matmul(
                        o[col:col + 32, n0:CW],
                        vaug[:, 192 * fs + 32 * h:192 * fs + 32 * h + 32],
                        et[:, n0:CW],
                        start=(fs == 0), stop=(fs == smax),
                        tile_position=(0, col), skip_group_check=True)

            for s in range(smax + 1):
                crossed = s >= 4 * c
                n0 = 128 * s - CW * c if crossed else 0
                with nc.named_scope(f"L{l}b{b}.att{c}"):
                    for h in range(NH):
                        spr_k, spr_q, row = ((kA, qA, 32 * h) if h < 4
                                             else (kB, qB, 32 * (h - 4)))
                        pqk = psum(128, CW)
                        nc.tensor.matmul(
                            pqk[:, 0:CW - n0],
                            spr_k[row:row + HS, 128 * s:128 * (s + 1)],
                            spr_q[row:row + HS, CW * c + n0:CW * (c + 1)],
                            start=True, stop=not (crossed and MASK_TRI),
                            tile_position=(row, 0),
                            skip_group_check=True)
                        if crossed and MASK_TRI:
                            nc.tensor.matmul(
                                pqk[:, 0:128], tri[:], idm[:],
                                start=False, stop=True,
                                tile_position=(0, 0), skip_group_check=True)
                        sch = SCHRAUD and h >= 4 and not ATT_F32
                        if sch:
                            # gpsimd Schraudolph exp: one add+convert; the AV
                            # matmul reads the int32's high halves as bf16 via
                            # a stride-2 bitcast view.
                            eti = ep.tile([128, CW], I32, tag="eti", bufs=10)
                            nc.vector.tensor_scalar(
                                out=eti[:, n0:CW], in0=pqk[:, 0:CW - n0],
                                scalar1=SCH_B, scalar2=None,
                                op0=mybir.AluOpType.add)
                            if crossed and not MASK_TRI:
                                nc.gpsimd.affine_select(
                                    out=eti[:, n0:n0 + 128],
                                    in_=eti[:, n0:n0 + 128],
                                    pattern=[[1, 128]],
                                    compare_op=mybir.AluOpType.is_ge,
                                    fill=0.0, base=0, channel_multiplier=-1)
                            et = eti[:].bitcast(BF16)[:, 1::2]
                            pend.append((s, h, et, n0))
                            continue
                        et = ep.tile([128, CW], EDT, tag="et")
                        if crossed and not MASK_TRI:
                            # diagonal band: exp into a scratch tile, then
                            # causal-select into et; rest exps directly
                            if CW - n0 > 128:
                                nc.scalar.activation(
                                    out=et[:, n0 + 128:CW],
                                    in_=pqk[:, 128:CW - n0],
                                    func=mybir.ActivationFunctionType.Exp)
                            etb = ep.tile([128, 128], EDT, tag="etb", bufs=12)
                            nc.scalar.activation(
                                out=etb[:], in_=pqk[:, 0:128],
                                func=mybir.ActivationFunctionType.Exp)
                            nc.gpsimd.affine_select(
                                out=et[:, n0:n0 + 128], in_=etb[:],
                                pattern=[[1, 128]],
                                compare_op=mybir.AluOpType.is_ge,
                                fill=0.0, base=0, channel_multiplier=-1)
                        else:
                            nc.scalar.activation(
                                out=et[:, n0:CW], in_=pqk[:, 0:CW - n0],
                                func=mybir.ActivationFunctionType.Exp)
                        pend.append((s, h, et[:], n0))
                    flush_av(NH if s < smax else 0)
                yield
            with nc.named_scope(f"L{l}b{b}.attep{c}"):
                # softmax denominators sit (replicated) on psum rows
                # col..col+16 of each band. Broadcast across the full 32-row
                # band with a constant selection matmul (sel[k,p]=1 iff
                # k == 32*(p//32)) via an SBUF bounce copy, reciprocal, then
                # scale. Rows col..col+16 of each ocat band become
                # sigma/sigma = 1.0 and hit zero rows of w_proj downstream.
                sden = tp.tile([128, CW], BF16, tag="sden")
                sdenB = tp.tile([64, CW], BF16, tag="sdenB")
                nc.vector.tensor_copy(out=sden[:], in_=oA[:])
                nc.vector.tensor_copy(out=sdenB[:], in_=oB[:])
                sbA = psum(128, CW)
                nc.tensor.matmul(sbA[:], selA[:], sden[:], start=True, stop=True)
                sbB = psum(64, CW)
                nc.tensor.matmul(sbB[:], selB[:], sdenB[:], start=True, stop=True)
                rbr = tp.tile([128, CW], F32, tag="rbr")
                rbrB = tp.tile([64, CW], F32, tag="rbrB")
                nc.vector.reciprocal_approx_fast(out=rbr[:], in_=sbA[:])
                nc.vector.reciprocal_approx_fast(out=rbrB[:], in_=sbB[:])
                for h in range(NH):
                    o, col = (oA, 32 * h) if h < 4 else (oB, 32 * (h - 4))
                    oc = ocatA if h < 4 else ocatB
                    rt = rbr if h < 4 else rbrB
                    nc.vector.tensor_mul(
                        out=oc[col:col + 32, CW * c:CW * (c + 1)],
                        in0=o[col:col + 32, :], in1=rt[col:col + 32, :])
            yield

    def post_gen(b, l):
        """proj + ln2 + ff for (b, l), yielding between op groups."""
        xT = XT[b]
        ocatA, ocatB = OC[b]
        for c in range(NCH):
            with nc.named_scope(f"L{l}b{b}.proj"):
                sl = slice(CW * c, CW * (c + 1))
                ps = psum(D, CW)
                nc.tensor.matmul(ps[:], WP[l][0][:], ocatA[:, sl],
                                 start=True, stop=False)
                nc.tensor.matmul(ps[:], WP[l][1][:], ocatB[:, sl],
                                 start=False, stop=True)
                nc.vector.tensor_scalar(out=ps[:], in0=ps[:], scalar1=BP[l][:],
                                        scalar2=None, op0=mybir.AluOpType.add)
                nc.vector.tensor_add(out=xT[:, sl], in0=xT[:, sl], in1=ps[:])
            yield
        yield from ln_gen(xT, L2G[l], L2B[l], "xn2", f"L{l}b{b}.ln2")
        xn2 = LNOUT["xn2"]
        h1 = [hp.tile([128, T], BF16, tag=f"h1_{f}", name=f"h1_{f}")
              for f in range(3)]
        for c in range(NCH):
            with nc.named_scope(f"L{l}b{b}.ff"):
                sl = slice(CW * c, CW * (c + 1))
                for f in range(3):
                    ps = psum(128, CW)
                    nc.tensor.matmul(ps[:], WF1[l][:, 128 * f:128 * (f + 1)],
                                     xn2[0:D, sl], start=True, stop=True)
                    nc.vector.tensor_scalar(
                        out=h1[f][:, sl], in0=ps[:], scalar1=BF1[l][f][:],
                        scalar2=0.0, op0=mybir.AluOpType.add,
                        op1=mybir.AluOpType.max)
            yield
        for c in range(NCH):
            with nc.named_scope(f"L{l}b{b}.ff"):
                sl = slice(CW * c, CW * (c + 1))
                ps = psum(D, CW)
                for f in range(3):
                    nc.tensor.matmul(ps[:], WF2[l][f][:], h1[f][:, sl],
                                     start=(f == 0), stop=(f == 2))
                nc.vector.tensor_scalar(out=ps[:], in0=ps[:], scalar1=BF2[l][:],
                                        scalar2=None, op0=mybir.AluOpType.add)
                nc.vector.tensor_add(out=xT[:, sl], in0=xT[:, sl], in1=ps[:])
            yield

    def head_gen(b):
        yield from ln_gen(XT[b], lnfg, lnfb, "xn1", f"head{b}")
        xnf = LNOUT["xn1"]
        for half in range(2):
            with nc.named_scope(f"head{b}"):
                for t in range(NT // 2 * half, NT // 2 * (half + 1)):
                    ps = psum(128, DA)
                    nc.tensor.matmul(ps[:], xnf[0:D, 128 * t:128 * (t + 1)],
                                     wact[:], start=True, stop=True)
                    osb = tp.tile([128, DA], F32, tag="osb")
                    nc.vector.tensor_add(out=osb[:], in0=ps[:], in1=bact_b[:])
                    nc.sync.dma_start(out=out[b, 128 * t:128 * (t + 1), :],
                                      in_=osb[:])
            yield

    def chain(*gens):
        for g in gens:
            yield from g

    def drain(g):
        for _ in g:
            pass

    def zip_emit(a, bgen):
        """Interleave two emission streams so the per-engine FIFOs see their
        ops finely interleaved (real software pipelining across the two batch
        elements)."""
        while True:
            sa = next(a, _STOP)
            sb = next(bgen, _STOP)
            if sa is _STOP and sb is _STOP:
                return

    _STOP = object()

    # Software-pipelined driver: while att(b) streams on PE/Scalar, the other
    # element's proj/ln2/ff and next-layer ln1/qkv (Vector-heavy) emit
    # interleaved, so every engine queue always holds ready work.
    drain(pre_gen(0, 0))
    carry = pre_gen(1, 0)  # b1 work that overlaps b0's attention
    for l in range(L):
        zip_emit(att_gen(0, l), carry)
        tail0 = pre_gen(0, l + 1) if l + 1 < L else head_gen(0)
        zip_emit(att_gen(1, l), chain(post_gen(0, l), tail0))
        carry = chain(post_gen(1, l),
                      pre_gen(1, l + 1) if l + 1 < L else head_gen(1))
    drain(carry)

    ctx.close()


_CACHED = None


def _pin_act_tables():
    # Pin every ACT function to the one set containing both Ln and Exp so the
    # table is loaded once instead of thrashing between sets per activation.
    from concourse import hw_specs
    import concourse.bacc as bacc_mod
    if getattr(hw_specs, "_act_tables_pinned", False):
        return
    orig = hw_specs.get_activation_tables

    def pinned(arch):
        t = orig(arch)
        keep = "natural_log_exp_and_others"
        return {n: (f if n == keep else set()) for n, f in t.items()}

    hw_specs.get_activation_tables = pinned
    if hasattr(bacc_mod, "get_activation_tables"):
        bacc_mod.get_activation_tables = pinned
    hw_specs._act_tables_pinned = True


def _get_nc():
    global _CACHED
    if _CACHED is None:
        _pin_act_tables()
        nc = bacc.Bacc("TRN2", target_bir_lowering=False, debug=False,
                       enable_asserts=False)
        build(nc)
        nc.compile()
        _CACHED = nc
    return _CACHED


def prep_weights(inputs):
    """Host-side layout/casting of the (replicated) weights."""
    f = {k: _dt(v) for k, v in inputs.items()}
    o = {}
    o["w_go"] = f["w_go"]
    o["b_go"] = f["b_go"]
    o["posT"] = np.ascontiguousarray(f["pos_emb"].T)
    wq, wk, wv, wpj = f["wq"], f["wk"], f["wv"], f["w_proj"]
    qa = np.zeros((L, D, 112), np.float32)
    qb = np.zeros((L, D, 48), np.float32)
    ka = np.zeros((L, D, 112), np.float32)
    kb = np.zeros((L, D, 48), np.float32)
    pa = np.zeros((L, 128, D), np.float32)
    pb = np.zeros((L, 64, D), np.float32)
    for h in range(NH):
        if h < 4:
            qa[:, :, 32 * h:32 * h + HS] = wq[:, h] * SCALE
            ka[:, :, 32 * h:32 * h + HS] = wk[:, h]
            # ocat rows col..col+16 are sigma/sigma = 1.0; o sits at +16
            pa[:, 32 * h + HS:32 * h + 32, :] = wpj[:, HS * h:HS * (h + 1), :]
        else:
            r = 32 * (h - 4)
            qsc = SCALE * (SCH_A if SCHRAUD else 1.0)
            qb[:, :, r:r + HS] = wq[:, h] * qsc
            kb[:, :, r:r + HS] = wk[:, h]
            pb[:, r + HS:r + 32, :] = wpj[:, HS * h:HS * (h + 1), :]
    o["wqa"] = qa.astype(NPBF)
    o["wqb"] = qb.astype(NPBF)
    o["wka"] = ka.astype(NPBF)
    o["wkb"] = kb.astype(NPBF)
    # wv_hd [L, D+1, NH*32]: per head, cols [32h:32h+16) come from the
    # constant-ones xn row (row D) -> all-ones output; cols [32h+16:32h+32)
    # are the V projection.
    wvp = np.zeros((L, D + 1, NH * 32), np.float32)
    for h in range(NH):
        wvp[:, 0:D, 32 * h + HS:32 * h + 32] = wv[:, h]
        wvp[:, D, 32 * h:32 * h + HS] = 1.0
    o["wv_hd"] = wvp.astype(NPBF)
    o["wpa"] = pa.astype(NPBF)
    o["wpb"] = pb.astype(NPBF)
    o["wf1"] = f["w_ff1"].astype(NPBF)
    o["wf2"] = f["w_ff2"].astype(NPBF)
    o["wact"] = f["w_act"].astype(NPBF)
    sa = np.zeros((128, 128), np.float32)
    sa[32 * (np.arange(128) // 32), np.arange(128)] = 1.0
    o["selA"] = sa.astype(NPBF)
    o["selB"] = np.ascontiguousarray(sa[:64, :64]).astype(NPBF)
    for k in ("ln1_g", "ln1_b", "ln2_g", "ln2_b", "b_proj", "b_ff1", "b_ff2",
              "lnf_g", "lnf_b", "b_act"):
        o[k] = f[k]
    return o, f


def kernel(**inputs):
    global LAST_RESULT
    nc = _get_nc()
    w, f = prep_weights(inputs)
    in_maps = []
    for i in range(NCORES):
        m = dict(w)
        m["goals"] = f["goals"][BPC * i:BPC * (i + 1)]
        m["obss"] = f["obss"][BPC * i:BPC * (i + 1)]
        in_maps.append(m)
    res = run_bass_kernel_spmd(nc, in_maps, core_ids=list(range(NCORES)))
    LAST_RESULT = res
    return np.concatenate([r["out"] for r in res.results], axis=0)
